# revision 2
# baseline (speedup 1.0000x reference)
"""DEQ block (Anderson acceleration, 6 iters, m=3) on 8 trn2 NeuronCores.

Data-parallel over batch: each core gets 512 of 4096 samples; W_z/W_x/b
replicated.  Per core the 512 samples are processed as two sequential
halves of 256 (2 m-tiles of 128) so all per-sample state stays SBUF
resident in fp32.  Matmuls run as float32r (FP22 reads, fp32 accumulate).

Per iteration i (z update, sample-major state):
  f   = tanh(z @ W_z + xwx)            PE (+identity-matmul xwx add) + ACT
  g   = f - z                          DVE scalar_tensor_tensor, in place
  u   = beta*g + z                     DVE scalar_tensor_tensor
  i<3:  z' = u  (buffer alias, no copy)
  i>=3: 2x2 regularized Anderson solve from 3 fresh dots
        P=<g,g> (ACT square+accum), Q1=<g,g1>, Q2=<g,g2> (DVE TTR),
        gram history terms reused from previous iterations' P/Q1;
        z' = s0*u + gamma1*u1 + gamma2*u2  (ACT scale + 2 DVE STT)
"""

import sys

sys.path.insert(0, "/opt/trn_rl_repo")

import numpy as np
from contextlib import ExitStack

import concourse.bass as bass
import concourse.tile as tile
from concourse import bacc, mybir, masks
from concourse import bass_utils

F32 = mybir.dt.float32
F32R = mybir.dt.float32r
F16 = mybir.dt.float16
ALU = mybir.AluOpType
ACTF = mybir.ActivationFunctionType

B, D = 4096, 2048
NCORES = 8
BC = B // NCORES          # 512 samples per core
NHALF = 2                 # sequential halves per core
CH = BC // NHALF          # 256 samples per half
MT = CH // 128            # 2 m-tiles per half
KT = D // 128             # 16 k-tiles
NT = D // 512             # 4 n-slices
RWZ = 4                   # W_z k-tiles kept SBUF resident; rest streamed
MAX_ITER, MAND = 6, 3
BETA, LAM = 0.8, 1e-4

_CACHE = {}

import os
NITER = int(os.environ.get("K_NITER", str(MAX_ITER)))   # iterations per half
NHALVES = int(os.environ.get("K_NHALVES", "2"))
FAKE_RES = int(os.environ.get("K_FAKE_RESIDENT", "0"))  # timing expt: no W stream


def _r(ap):
    return ap.bitcast(F32R)


def _build():
    nc = bacc.Bacc("TRN2", target_bir_lowering=False, debug=False,
                   num_devices=NCORES)

    x_d = nc.dram_tensor("x", [BC, D], F32, kind="ExternalInput").ap()
    wz_d = nc.dram_tensor("W_z", [D, D], F32, kind="ExternalInput").ap()
    wx_d = nc.dram_tensor("W_x", [D, D], F32, kind="ExternalInput").ap()
    b_d = nc.dram_tensor("b", [D], F32, kind="ExternalInput").ap()
    out_d = nc.dram_tensor("z_out", [BC, D], F32, kind="ExternalOutput").ap()
    # staging for half-1's xwx (computed in phase 0, reloaded at half 1)
    xwx1_d = nc.dram_tensor("xwx1_stage", [MT, 128, D], F16, kind="Internal").ap()

    with tile.TileContext(nc) as tc, ExitStack() as ctx:
        # ---------------- pools ----------------
        state = ctx.enter_context(tc.tile_pool(name="state", bufs=1))

        def persist(shape, nm):
            return state.tile(shape, F32, tag=nm, name=nm)

        wz16 = [state.tile([128, D], F16, tag=f"wz16_{k}", name=f"wz16_{k}")
                for k in range(KT)]
        zbuf = [persist([128, D], f"zbuf{m}") for m in range(MT)]
        gsl = [[persist([128, D], f"g{j}_{m}") for m in range(MT)]
               for j in range(3)]
        usl = [[persist([128, D], f"u{j}_{m}") for m in range(MT)]
               for j in range(3)]
        xwx = [state.tile([128, D], F16, tag=f"xwx{m}", name=f"xwx{m}")
               for m in range(MT)]
        ident = persist([128, 128], "ident")

        wpool = ctx.enter_context(tc.tile_pool(name="wstream", bufs=2))
        ztpool = ctx.enter_context(tc.tile_pool(name="ztp", bufs=33))
        dots = ctx.enter_context(tc.tile_pool(name="dots", bufs=40))
        typs = ctx.enter_context(tc.tile_pool(name="tpsum", bufs=3, space="PSUM"))
        yps = ctx.enter_context(tc.tile_pool(name="ypsum", bufs=4, space="PSUM"))

        pdump = state.tile([128, 512], F32, tag="pdump", name="pdump")
        qdump = state.tile([128, 512], F32, tag="qdump", name="qdump")
        masks.make_identity(nc, ident[:])
        identh = state.tile([128, 128], F16, tag="identh", name="identh")
        nc.vector.tensor_copy(identh[:], ident[:])
        rid = ident[:]          # fp32, rhs of fp32 transposes
        ridh = identh[:]        # fp16, lhsT of the xwx identity-matmul

        # W_z: DMA fp32 rows in, round to resident fp16 tiles on DVE
        for k in range(KT):
            for j in range(2):
                wrow = wpool.tile([128, 1024], F32, tag="w", name=f"wl{k}_{j}")
                nc.sync.dma_start(wrow[:], wz_d[k * 128:(k + 1) * 128,
                                               j * 1024:(j + 1) * 1024])
                nc.vector.tensor_copy(wz16[k][:, j * 1024:(j + 1) * 1024],
                                      wrow[:])

        def uw(j, ap):
            # usl[0]/usl[1] memlocs are fp32r-consumed (XT backing): every
            # engine write into them must round to fp32r for the verifier
            return _r(ap) if j in (0, 1) else ap

        def stt(out, in0, scalar, in1, op0, op1):
            nc.vector.scalar_tensor_tensor(
                out=out, in0=in0, scalar=scalar, in1=in1, op0=op0, op1=op1)

        # XT backing: 16 transposed-x k-rows [128, 512] live inside the
        # (not yet used) u-ring tiles during phase 0.
        def xt_sl(k, q):
            back = [usl[0][0], usl[0][1], usl[1][0], usl[1][1]][k // 4]
            off = (k % 4) * 512 + q * 128
            return back[:, off:off + 128]

        # ---------------- phase 0: xwx for all 4 quarter-tiles ----------------
        for q in range(4):
            xs = []
            for h2 in range(2):
                xst = wpool.tile([128, 1024], F32, tag="w", name=f"xst{q}_{h2}")
                nc.sync.dma_start(xst[:], x_d[q * 128:(q + 1) * 128,
                                               h2 * 1024:(h2 + 1) * 1024])
                xs.append(xst)
            for k in range(KT):
                tp = typs.tile([128, 128], F32, tag="tp", name=f"xtp{q}_{k}")
                src = xs[k // 8][:, (k % 8) * 128:(k % 8 + 1) * 128]
                nc.tensor.transpose(tp[:], src, rid)
                nc.scalar.copy(_r(xt_sl(k, q)), tp[:])


        b2d = b_d.rearrange("(p n) -> p n", p=1)
        for n in range(NT):
            b1 = wpool.tile([1, 512], F32, tag="w", name=f"b1_{n}")
            nc.sync.dma_start(b1[:], b2d[:, n * 512:(n + 1) * 512])
            bsl = wpool.tile([128, 512], F32, tag="w", name=f"bsl{n}")
            nc.gpsimd.partition_broadcast(bsl[:], b1[:])
            ps = [yps.tile([128, 512], F32, tag="yp", name=f"xwps{n}_{q}") for q in range(4)]
            for k in range(KT):
                wt = wpool.tile([128, 512], F32R, tag="w", name=f"wx{n}_{k}")
                nc.sync.dma_start(wt[:], _r(wx_d[k * 128:(k + 1) * 128,
                                                 n * 512:(n + 1) * 512]))
                for q in range(4):
                    nc.tensor.matmul(ps[q][:], _r(xt_sl(k, q)), wt[:],
                                     start=(k == 0), stop=(k == KT - 1))
            for q in range(4):
                if q < MT:
                    dst = xwx[q][:, n * 512:(n + 1) * 512]
                else:
                    dst = zbuf[q - MT].bitcast(F16)[:, n * 512:(n + 1) * 512]
                stt(dst, ps[q][:], 1.0, bsl[:], ALU.mult, ALU.add)
        for m in range(MT):
            nc.sync.dma_start(xwx1_d[m],
                              zbuf[m].bitcast(F16)[:, 0:D])

        # ---------------- per-half iterations ----------------
        def emit_half(h):
            if h == 1:
                for m in range(MT):
                    nc.sync.dma_start(xwx[m][:], xwx1_d[m])

            hist = {}  # (kind, i, m) -> [128,1] ap

            # iteration 0: z=0 -> g0 = tanh(xwx), u0 = beta*g0, z1 aliases u0
            for m in range(MT):
                nc.scalar.activation(gsl[0][m][:], xwx[m][:], ACTF.Tanh)
                nc.vector.tensor_scalar_mul(_r(usl[0][m][:]), gsl[0][m][:], BETA)

            for i in range(1, NITER):
                gi, ui = gsl[i % 3], usl[i % 3]
                g1, g2 = gsl[(i - 1) % 3], gsl[(i - 2) % 3]
                u1, u2 = usl[(i - 1) % 3], usl[(i - 2) % 3]
                zc = usl[i - 1] if i <= 3 else zbuf  # current z (alias)

                # transpose z into lhsT k-tiles
                zt = {}
                for m in range(MT):
                    for k in range(KT):
                        tp = typs.tile([128, 128], F32, tag="tp", name=f"tp{h}_{i}_{m}_{k}")
                        nc.tensor.transpose(
                            tp[:], zc[m][:, k * 128:(k + 1) * 128], rid)
                        zs = ztpool.tile([128, 128], F16, tag="zt",
                                         name=f"zt{h}_{i}_{m}_{k}")
                        nc.scalar.copy(zs[:], tp[:])
                        zt[m, k] = zs

                # matmul + xwx add + tanh, n-slice major
                for n in range(NT):
                    ps = [yps.tile([128, 512], F32, tag="yp", name=f"yp{h}_{i}_{n}_{m}")
                          for m in range(MT)]
                    for k in range(KT):
                        wsl = wz16[k][:, n * 512:(n + 1) * 512]
                        for m in range(MT):
                            nc.tensor.matmul(ps[m][:], zt[m, k][:], wsl,
                                             start=(k == 0), stop=False)
                    for m in range(MT):
                        nc.tensor.matmul(ps[m][:], ridh,
                                         xwx[m][:, n * 512:(n + 1) * 512],
                                         start=False, stop=True)
                        nc.scalar.activation(gi[m][:, n * 512:(n + 1) * 512],
                                             ps[m][:], ACTF.Tanh)

                for m in range(MT):
                    # g = f - z ; u = beta*g + z
                    stt(gi[m][:], gi[m][:], 1.0, zc[m][:], ALU.mult, ALU.subtract)
                    stt(uw(i % 3, ui[m][:]), gi[m][:], BETA, zc[m][:], ALU.mult, ALU.add)

                    # P = <g,g> on ACT (square + accum), dumped to PSUM
                    pc = dots.tile([128, 4], F32, tag="d", name=f"pc{h}_{i}_{m}")
                    for c in range(4):
                        nc.scalar.activation(pdump[:],
                                             gi[m][:, c * 512:(c + 1) * 512],
                                             ACTF.Square,
                                             accum_out=pc[:, c:c + 1])
                    pp = dots.tile([128, 1], F32, tag="d", name=f"p{h}_{i}_{m}")
                    nc.vector.tensor_reduce(pp[:], pc[:], mybir.AxisListType.X,
                                            ALU.add)
                    hist["P", i, m] = pp

                    def ttr_dot(gh, nm):
                        qc = dots.tile([128, 4], F32, tag="d", name=f"{nm}c")
                        for c in range(4):
                            nc.vector.scalar_tensor_tensor(
                                out=qdump[:],
                                in0=gi[m][:, c * 512:(c + 1) * 512],
                                scalar=1.0,
                                in1=gh[m][:, c * 512:(c + 1) * 512],
                                op0=ALU.mult, op1=ALU.mult,
                                accum_out=qc[:, c:c + 1])
                        qq = dots.tile([128, 1], F32, tag="d", name=nm)
                        nc.vector.tensor_reduce(qq[:], qc[:],
                                                mybir.AxisListType.X, ALU.add)
                        return qq

                    if i >= 2:
                        hist["Q1", i, m] = ttr_dot(g1, f"q1_{h}_{i}_{m}")
                    if i >= 3:
                        q2t = ttr_dot(g2, f"q2_{h}_{i}_{m}")

                        P = hist["P", i, m][:]
                        Q1 = hist["Q1", i, m][:]
                        Q2 = q2t[:]
                        S11 = hist["P", i - 1, m][:]
                        S12 = hist["Q1", i - 1, m][:]
                        S22 = hist["P", i - 2, m][:]

                        def tnew(nm):
                            return dots.tile([128, 1], F32, tag="d",
                                             name=f"{nm}_{h}_{i}_{m}")[:]

                        def ts(out, in0, s1, s2, op0, op1=None):
                            nc.vector.tensor_scalar(out, in0, s1, s2, op0,
                                                    *( [op1] if op1 else []))

                        def aff(out, in_, scale, bias):
                            nc.scalar.activation(out, in_, ACTF.Identity,
                                                 bias=bias, scale=scale)

                        r0 = tnew("r0"); ts(r0, P, Q1, None, ALU.subtract)
                        r1 = tnew("r1"); ts(r1, P, Q2, None, ALU.subtract)
                        a1 = tnew("a1"); aff(a1, Q1, -2.0, S11)
                        av = tnew("av"); ts(av, a1, LAM, P, ALU.add, ALU.add)
                        d1 = tnew("d1"); aff(d1, Q2, -2.0, S22)
                        dv = tnew("dv"); ts(dv, d1, LAM, P, ALU.add, ALU.add)
                        b1 = tnew("b1"); aff(b1, Q2, -1.0, S12)
                        bv = tnew("bv"); ts(bv, b1, r0, None, ALU.add)
                        t4 = tnew("t4"); aff(t4, av, dv, 0.0)
                        t5 = tnew("t5"); nc.scalar.square(t5, bv)
                        det = tnew("det")
                        ts(det, t4, 1e-8, t5, ALU.add, ALU.subtract)
                        idet = tnew("idet"); nc.vector.reciprocal(idet, det)
                        g1a = tnew("g1a"); aff(g1a, dv, r0, 0.0)
                        g1b = tnew("g1b"); ts(g1b, bv, r1, None, ALU.mult)
                        g1c = tnew("g1c"); ts(g1c, g1a, g1b, None, ALU.subtract)
                        gam1 = tnew("gam1"); ts(gam1, g1c, idet, None, ALU.mult)
                        g2a = tnew("g2a"); aff(g2a, av, r1, 0.0)
                        g2b = tnew("g2b"); ts(g2b, bv, r0, None, ALU.mult)
                        g2c = tnew("g2c"); ts(g2c, g2a, g2b, None, ALU.subtract)
                        gam2 = tnew("gam2"); ts(gam2, g2c, idet, None, ALU.mult)
                        s0a = tnew("s0a")
                        ts(s0a, gam1, -1.0, gam2, ALU.mult, ALU.subtract)
                        s0 = tnew("s0"); aff(s0, s0a, 1.0, 1.0)

                        # z' = s0*u + gam1*u1 + gam2*u2 (u2 slot is scratch)
                        ju = (i - 2) % 3
                        nc.scalar.mul(uw(ju, u2[m][:]), u2[m][:], gam2)
                        stt(uw(ju, u2[m][:]), u1[m][:], gam1, u2[m][:],
                            ALU.mult, ALU.add)
                        stt(zbuf[m][:], ui[m][:], s0, u2[m][:],
                            ALU.mult, ALU.add)

            for m in range(MT):
                q = h * MT + m
                nc.sync.dma_start(out_d[q * 128:(q + 1) * 128, :], zbuf[m][:])

        emit_half(0)
        if NHALVES > 1:
            emit_half(1)

    nc.compile()
    return nc


def kernel(x_input, W_z, W_x, b):
    x_input = np.ascontiguousarray(x_input, dtype=np.float32)
    W_z = np.ascontiguousarray(W_z, dtype=np.float32)
    W_x = np.ascontiguousarray(W_x, dtype=np.float32)
    b = np.ascontiguousarray(b, dtype=np.float32)

    if "nc" not in _CACHE:
        _CACHE["nc"] = _build()
    nc = _CACHE["nc"]

    in_maps = [{
        "x": x_input[i * BC:(i + 1) * BC],
        "W_z": W_z, "W_x": W_x, "b": b,
    } for i in range(NCORES)]

    res = bass_utils.run_bass_kernel_spmd(nc, in_maps,
                                          core_ids=list(range(NCORES)),
                                          tmpdir=os.environ.get("K_TMPDIR"))
    _CACHE["res"] = res
    out = np.concatenate([res.results[i]["z_out"] for i in range(NCORES)],
                         axis=0)
    return out.astype(np.float32)



# revision 8
# speedup vs baseline: 1.3303x; 1.3303x over previous
"""DEQ block (Anderson acceleration, 6 iters, m=3) on 8 trn2 NeuronCores.

Data-parallel over batch: each core gets 512 of 4096 samples; W_z/W_x/b
replicated.  Single pass: all 512 samples stay SBUF-resident as 4 m-tiles
of 128 in fp16.  z shares the u-ring slots (z_{i+1} = s0*u_i + g1*u_{i-1}
+ g2*u_{i-2} overwrites the retiring u slot).

Per iteration: z^T k-tiles come from DMA-xbar transposes (no PE transpose
pass), matmuls are fp16 N=1024 accumulating over 16 k-tiles in PSUM with
xwx added via an identity matmul, tanh drains PSUM->fp16 g on ACT, the
per-sample dots P/Q1/Q2 use ACT-square / DVE tensor_tensor_reduce accums,
and the 2x2 regularized Anderson solve runs as ~21 fused [128,1] DVE ops.
"""

import sys

sys.path.insert(0, "/opt/trn_rl_repo")

import os
import numpy as np
from contextlib import ExitStack

import concourse.bass as bass
import concourse.tile as tile
from concourse import bacc, mybir, masks
from concourse import bass_utils

F32 = mybir.dt.float32
F16 = mybir.dt.float16
ALU = mybir.AluOpType
ACTF = mybir.ActivationFunctionType

B, D = 4096, 2048
NCORES = 8
BC = B // NCORES          # 512 samples per core
MT = BC // 128            # 4 m-tiles
KT = D // 128             # 16 k-tiles
NH = 2                    # n-halves for DMA/convert chunks
NW = D // NH              # 1024-wide loads
NS = 4                    # n-slices for matmuls
SW = D // NS              # 512-wide matmuls (sliced operands need N<=512)
MAX_ITER = 6
BETA, LAM = 0.8, 1e-4

NITER = int(os.environ.get("K_NITER", str(MAX_ITER)))

_CACHE = {}


def _build():
    nc = bacc.Bacc("TRN2", target_bir_lowering=False, debug=False,
                   num_devices=NCORES)

    x_d = nc.dram_tensor("x", [BC, D], F32, kind="ExternalInput").ap()
    wz_d = nc.dram_tensor("W_z", [D, D], F32, kind="ExternalInput").ap()
    wx_d = nc.dram_tensor("W_x", [D, D], F32, kind="ExternalInput").ap()
    b_d = nc.dram_tensor("b", [D], F32, kind="ExternalInput").ap()
    out_d = nc.dram_tensor("z_out", [BC, D], F32, kind="ExternalOutput").ap()

    with tile.TileContext(nc) as tc, ExitStack() as ctx:
        state = ctx.enter_context(tc.tile_pool(name="state", bufs=1))

        wz16 = state.tile([128, KT * D], F16, tag="wz16", name="wz16")
        gsl = [[state.tile([128, D], F16, tag=f"g{j}_{m}", name=f"g{j}_{m}")
                for m in range(MT)] for j in range(3)]
        usl = [[state.tile([128, D], F16, tag=f"u{j}_{m}", name=f"u{j}_{m}")
                for m in range(MT)] for j in range(3)]
        xwx = [state.tile([128, D], F16, tag=f"xwx{m}", name=f"xwx{m}")
               for m in range(MT)]
        zt = [state.tile([128, D], F16, tag=f"zt{m}", name=f"zt{m}")
              for m in range(MT)]
        identh = state.tile([128, 128], F16, tag="idh", name="idh")

        scratch = ctx.enter_context(tc.tile_pool(name="scratch", bufs=2))
        wtmp = ctx.enter_context(tc.tile_pool(name="wtmp", bufs=2))
        dots = ctx.enter_context(tc.tile_pool(name="dots", bufs=64))
        yps = ctx.enter_context(tc.tile_pool(name="yps", bufs=8, space="PSUM"))

        id32 = scratch.tile([128, NW], F32, tag="s", name="id32")
        masks.make_identity(nc, id32[:, 0:128])
        nc.vector.tensor_copy(identh[:], id32[:, 0:128])

        def stt(out, in0, scalar, in1, op0, op1):
            nc.vector.scalar_tensor_tensor(
                out=out, in0=in0, scalar=scalar, in1=in1, op0=op0, op1=op1)

        def ts(out, in0, s1, s2, op0, op1=None):
            nc.vector.tensor_scalar(out, in0, s1, s2, op0,
                                    *([op1] if op1 is not None else []))

        def dnew(nm):
            return dots.tile([128, 1], F32, tag="d", name=nm)[:]

        def transpose_into_zt(m, src):
            for k in range(KT):
                nc.sync.dma_start(zt[m][:, k * 128:(k + 1) * 128],
                                  src[:, k * 128:(k + 1) * 128],
                                  transpose=True)

        # ---------------- b -> broadcast fp16 ----------------
        # staged in the (not yet used) g1 ring slot of m=0
        b2d = b_d.rearrange("(p n) -> p n", p=1)
        b16 = gsl[1][0]
        for h in range(NH):
            b1 = scratch.tile([1, NW], F32, tag="s", name=f"b1_{h}")
            nc.sync.dma_start(b1[:], b2d[:, h * NW:(h + 1) * NW])
            bsl = scratch.tile([128, NW], F32, tag="s", name=f"bsl{h}")
            nc.gpsimd.partition_broadcast(bsl[:], b1[:])
            nc.vector.tensor_copy(b16[:, h * NW:(h + 1) * NW], bsl[:])

        # ---------------- x load, fp16, transpose ----------------
        # x16 staged in the (not yet used) g2 ring slots
        for m in range(MT):
            for h in range(NH):
                xs = scratch.tile([128, NW], F32, tag="s", name=f"xs{m}_{h}")
                nc.sync.dma_start(xs[:], x_d[m * 128:(m + 1) * 128,
                                             h * NW:(h + 1) * NW])
                nc.vector.tensor_copy(gsl[2][m][:, h * NW:(h + 1) * NW],
                                      xs[:])
            transpose_into_zt(m, gsl[2][m][:])

        # ---------------- xwx = x @ W_x + b ----------------
        for nh in range(NH):
            ps = [[yps.tile([128, SW], F32, tag="yp", name=f"xwps{nh}_{m}_{j}")
                   for j in range(2)] for m in range(MT)]
            for k in range(KT):
                wxs = scratch.tile([128, NW], F32, tag="s", name=f"wxs{nh}_{k}")
                nc.sync.dma_start(wxs[:], wx_d[k * 128:(k + 1) * 128,
                                               nh * NW:(nh + 1) * NW])
                w16 = wtmp.tile([128, NW], F16, tag="w", name=f"wx16{nh}_{k}")
                nc.vector.tensor_copy(w16[:], wxs[:])
                for m in range(MT):
                    for j in range(2):
                        nc.tensor.matmul(
                            ps[m][j][:], zt[m][:, k * 128:(k + 1) * 128],
                            w16[:, j * SW:(j + 1) * SW],
                            start=(k == 0), stop=(k == KT - 1))
            for m in range(MT):
                for j in range(2):
                    sl = slice(nh * NW + j * SW, nh * NW + (j + 1) * SW)
                    stt(xwx[m][:, sl], ps[m][j][:], 1.0,
                        b16[:, sl], ALU.mult, ALU.add)

        # ---------------- W_z load + fp16 convert ----------------
        for k in range(KT):
            for h in range(NH):
                wzs = scratch.tile([128, NW], F32, tag="s", name=f"wzs{k}_{h}")
                nc.sync.dma_start(wzs[:], wz_d[k * 128:(k + 1) * 128,
                                               h * NW:(h + 1) * NW])
                nc.vector.tensor_copy(
                    wz16[:, k * D + h * NW:k * D + (h + 1) * NW], wzs[:])

        # ---------------- iteration 0: z=0 ----------------
        # g0 = tanh(xwx); u0 = beta*g0; z1 aliases u0
        for m in range(MT):
            nc.scalar.activation(gsl[0][m][:], xwx[m][:], ACTF.Tanh)
            nc.vector.tensor_scalar_mul(usl[0][m][:], gsl[0][m][:], BETA)
            transpose_into_zt(m, usl[0][m][:])

        # z_i lives in u-ring slot zslot[i]
        zslot = [None, 0, 1, 2, 1, 2]
        hist = {}  # (kind, i, m) -> [128,1] ap

        for i in range(1, NITER):
            gi = gsl[i % 3]
            for m in range(MT):
                # matmul: y = z @ W_z + xwx, f = tanh(y)
                for ns in range(NS):
                    ps = yps.tile([128, SW], F32, tag="yp",
                                  name=f"yp{i}_{m}_{ns}")
                    sl = slice(ns * SW, (ns + 1) * SW)
                    nc.tensor.matmul(ps[:], identh[:], xwx[m][:, sl],
                                     start=True, stop=False)
                    for k in range(KT):
                        nc.tensor.matmul(
                            ps[:], zt[m][:, k * 128:(k + 1) * 128],
                            wz16[:, k * D + ns * SW:k * D + (ns + 1) * SW],
                            start=False, stop=(k == KT - 1))
                    nc.scalar.activation(gi[m][:, sl], ps[:], ACTF.Tanh)

                z = usl[zslot[i]][m]
                g = gi[m]
                # g = f - z ; u_i = beta*g + z (u_i slot == z slot for i>=4)
                stt(g[:], g[:], 1.0, z[:], ALU.mult, ALU.subtract)
                stt(usl[i % 3][m][:], g[:], BETA, z[:], ALU.mult, ALU.add)

                # P = <g,g> on ACT (square + accum), dump into zt[m]
                pp = dnew(f"p{i}_{m}")
                nc.scalar.activation(zt[m][:], g[:], ACTF.Square,
                                     accum_out=pp)
                hist["P", i, m] = pp

                if i >= 2:
                    q1 = dnew(f"q1_{i}_{m}")
                    nc.vector.scalar_tensor_tensor(
                        out=zt[m][:], in0=g[:], scalar=1.0,
                        in1=gsl[(i - 1) % 3][m][:],
                        op0=ALU.mult, op1=ALU.mult, accum_out=q1)
                    hist["Q1", i, m] = q1
                if i >= 3:
                    q2 = dnew(f"q2_{i}_{m}")
                    nc.vector.scalar_tensor_tensor(
                        out=zt[m][:], in0=g[:], scalar=1.0,
                        in1=gsl[(i - 2) % 3][m][:],
                        op0=ALU.mult, op1=ALU.mult, accum_out=q2)

                if i < 3:
                    # z_{i+1} = u_i (alias); feed next iteration's lhsT
                    transpose_into_zt(m, usl[i % 3][m][:])
                    continue

                # ---- 2x2 regularized Anderson solve ----
                P = hist["P", i, m]
                Q1 = hist["Q1", i, m]
                Q2 = q2
                S11 = hist["P", i - 1, m]
                S12 = hist["Q1", i - 1, m]
                S22 = hist["P", i - 2, m]

                r0 = dnew(f"r0_{i}_{m}"); ts(r0, P, Q1, None, ALU.subtract)
                r1 = dnew(f"r1_{i}_{m}"); ts(r1, P, Q2, None, ALU.subtract)
                a1 = dnew(f"a1_{i}_{m}")
                ts(a1, Q1, -2.0, S11, ALU.mult, ALU.add)
                av = dnew(f"av_{i}_{m}")
                ts(av, a1, LAM, P, ALU.add, ALU.add)
                d1 = dnew(f"d1_{i}_{m}")
                ts(d1, Q2, -2.0, S22, ALU.mult, ALU.add)
                dv = dnew(f"dv_{i}_{m}")
                ts(dv, d1, LAM, P, ALU.add, ALU.add)
                b0 = dnew(f"b0_{i}_{m}"); ts(b0, Q1, Q2, None, ALU.add)
                b1t = dnew(f"b1_{i}_{m}"); ts(b1t, S12, P, None, ALU.add)
                bv = dnew(f"bv_{i}_{m}")
                ts(bv, b1t, b0, None, ALU.subtract)
                t4 = dnew(f"t4_{i}_{m}"); ts(t4, av, dv, None, ALU.mult)
                t5 = dnew(f"t5_{i}_{m}"); ts(t5, bv, bv, None, ALU.mult)
                det = dnew(f"det_{i}_{m}")
                ts(det, t4, t5, 1e-8, ALU.subtract, ALU.add)
                idet = dnew(f"idet_{i}_{m}")
                nc.vector.reciprocal(idet, det)
                g1a = dnew(f"g1a_{i}_{m}"); ts(g1a, dv, r0, None, ALU.mult)
                g1b = dnew(f"g1b_{i}_{m}"); ts(g1b, bv, r1, None, ALU.mult)
                gam1 = dnew(f"gam1_{i}_{m}")
                ts(gam1, g1a, g1b, idet, ALU.subtract, ALU.mult)
                g2a = dnew(f"g2a_{i}_{m}"); ts(g2a, av, r1, None, ALU.mult)
                g2b = dnew(f"g2b_{i}_{m}"); ts(g2b, bv, r0, None, ALU.mult)
                gam2 = dnew(f"gam2_{i}_{m}")
                ts(gam2, g2a, g2b, idet, ALU.subtract, ALU.mult)
                s0a = dnew(f"s0a_{i}_{m}"); ts(s0a, gam1, gam2, None, ALU.add)
                s0 = dnew(f"s0_{i}_{m}")
                ts(s0, s0a, -1.0, 1.0, ALU.mult, ALU.add)

                # z_{i+1} = s0*u_i + gam1*u_{i-1} + gam2*u_{i-2},
                # into u_{i-2}'s slot
                u_i = usl[i % 3][m]
                u_1 = usl[(i - 1) % 3][m]
                u_2 = usl[(i - 2) % 3][m]
                if i < NITER - 1:
                    nc.scalar.mul(u_2[:], u_2[:], gam2)
                    stt(u_2[:], u_1[:], gam1, u_2[:], ALU.mult, ALU.add)
                    stt(u_2[:], u_i[:], s0, u_2[:], ALU.mult, ALU.add)
                    transpose_into_zt(m, u_2[:])
                else:
                    for h in range(NH):
                        sl = slice(h * NW, (h + 1) * NW)
                        o32 = scratch.tile([128, NW], F32, tag="s",
                                           name=f"o32_{m}_{h}")
                        nc.scalar.activation(o32[:], u_2[:, sl],
                                             ACTF.Identity, scale=gam2)
                        stt(o32[:], u_1[:, sl], gam1, o32[:],
                            ALU.mult, ALU.add)
                        stt(o32[:], u_i[:, sl], s0, o32[:],
                            ALU.mult, ALU.add)
                        nc.sync.dma_start(
                            out_d[m * 128:(m + 1) * 128, sl], o32[:])

        if NITER < MAX_ITER:
            # debug runs: dump whatever z slot is current
            zz = usl[zslot[NITER] if NITER >= 1 else 0]
            for m in range(MT):
                for h in range(NH):
                    sl = slice(h * NW, (h + 1) * NW)
                    o32 = scratch.tile([128, NW], F32, tag="s",
                                       name=f"oz{m}_{h}")
                    nc.vector.tensor_copy(o32[:], zz[m][:, sl])
                    nc.sync.dma_start(out_d[m * 128:(m + 1) * 128, sl],
                                      o32[:])

    nc.compile()
    return nc


def kernel(x_input, W_z, W_x, b):
    x_input = np.ascontiguousarray(x_input, dtype=np.float32)
    W_z = np.ascontiguousarray(W_z, dtype=np.float32)
    W_x = np.ascontiguousarray(W_x, dtype=np.float32)
    b = np.ascontiguousarray(b, dtype=np.float32)

    if "nc" not in _CACHE:
        _CACHE["nc"] = _build()
    nc = _CACHE["nc"]

    in_maps = [{
        "x": x_input[i * BC:(i + 1) * BC],
        "W_z": W_z, "W_x": W_x, "b": b,
    } for i in range(NCORES)]

    res = bass_utils.run_bass_kernel_spmd(nc, in_maps,
                                          core_ids=list(range(NCORES)),
                                          tmpdir=os.environ.get("K_TMPDIR"))
    _CACHE["res"] = res
    out = np.concatenate([res.results[i]["z_out"] for i in range(NCORES)],
                         axis=0)
    return out.astype(np.float32)


# revision 9
# speedup vs baseline: 1.8019x; 1.3545x over previous
"""DEQ block (Anderson acceleration, 6 iters, m=3) on 8 trn2 NeuronCores.

Data-parallel over batch: each core gets 512 of 4096 samples; W_z/W_x/b
replicated.  Single pass: all 512 samples stay SBUF-resident as 4 m-tiles
of 128 in fp16.  z shares the u-ring slots (z_{i+1} = s0*u_i + g1*u_{i-1}
+ g2*u_{i-2} overwrites the retiring u slot).

Per iteration: z^T k-tiles come from DMA-xbar transposes (no PE transpose
pass), matmuls are fp16 N=1024 accumulating over 16 k-tiles in PSUM with
xwx added via an identity matmul, tanh drains PSUM->fp16 g on ACT, the
per-sample dots P/Q1/Q2 use ACT-square / DVE tensor_tensor_reduce accums,
and the 2x2 regularized Anderson solve runs as ~21 fused [128,1] DVE ops.
"""

import sys

sys.path.insert(0, "/opt/trn_rl_repo")

import os
import numpy as np
from contextlib import ExitStack

import concourse.bass as bass
import concourse.tile as tile
from concourse import bacc, mybir, masks
from concourse import bass_utils

F32 = mybir.dt.float32
F16 = mybir.dt.float16
ALU = mybir.AluOpType
ACTF = mybir.ActivationFunctionType

B, D = 4096, 2048
NCORES = 8
BC = B // NCORES          # 512 samples per core
MT = BC // 128            # 4 m-tiles
KT = D // 128             # 16 k-tiles
NH = 2                    # n-halves for DMA/convert chunks
NW = D // NH              # 1024-wide loads
NS = 4                    # n-slices for matmuls
SW = D // NS              # 512-wide matmuls (sliced operands need N<=512)
MAX_ITER = 6
BETA, LAM = 0.8, 1e-4

NITER = int(os.environ.get("K_NITER", str(MAX_ITER)))

_CACHE = {}


def _build():
    nc = bacc.Bacc("TRN2", target_bir_lowering=False, debug=False,
                   num_devices=NCORES)

    x_d = nc.dram_tensor("x", [BC, D], F32, kind="ExternalInput").ap()
    wz_d = nc.dram_tensor("W_z", [D, D], F32, kind="ExternalInput").ap()
    wx_d = nc.dram_tensor("W_x", [D, D], F32, kind="ExternalInput").ap()
    b_d = nc.dram_tensor("b", [D], F32, kind="ExternalInput").ap()
    out_d = nc.dram_tensor("z_out", [BC, D], F32, kind="ExternalOutput").ap()

    with tile.TileContext(nc) as tc, ExitStack() as ctx:
        state = ctx.enter_context(tc.tile_pool(name="state", bufs=1))

        wz16 = state.tile([128, KT * D], F16, tag="wz16", name="wz16")
        gsl = [[state.tile([128, D], F16, tag=f"g{j}_{m}", name=f"g{j}_{m}")
                for m in range(MT)] for j in range(3)]
        usl = [[state.tile([128, D], F16, tag=f"u{j}_{m}", name=f"u{j}_{m}")
                for m in range(MT)] for j in range(3)]
        xwx = [state.tile([128, D], F16, tag=f"xwx{m}", name=f"xwx{m}")
               for m in range(MT)]
        zt = [state.tile([128, D], F16, tag=f"zt{m}", name=f"zt{m}")
              for m in range(MT)]
        identh = state.tile([128, 128], F16, tag="idh", name="idh")

        scratch = ctx.enter_context(tc.tile_pool(name="scratch", bufs=2))
        wtmp = ctx.enter_context(tc.tile_pool(name="wtmp", bufs=2))
        dots = ctx.enter_context(tc.tile_pool(name="dots", bufs=64))
        yps = ctx.enter_context(tc.tile_pool(name="yps", bufs=6, space="PSUM"))
        typs = ctx.enter_context(tc.tile_pool(name="typs", bufs=2, space="PSUM"))

        id32 = scratch.tile([128, NW], F32, tag="s", name="id32")
        masks.make_identity(nc, id32[:, 0:128])
        nc.vector.tensor_copy(identh[:], id32[:, 0:128])

        def stt(out, in0, scalar, in1, op0, op1):
            nc.vector.scalar_tensor_tensor(
                out=out, in0=in0, scalar=scalar, in1=in1, op0=op0, op1=op1)

        def ts(out, in0, s1, s2, op0, op1=None):
            nc.vector.tensor_scalar(out, in0, s1, s2, op0,
                                    *([op1] if op1 is not None else []))

        def dnew(nm):
            return dots.tile([128, 1], F32, tag="d", name=nm)[:]

        def transpose_into_zt(m, src, nm):
            for k in range(KT):
                tp = typs.tile([128, 128], F16, tag="tp", name=f"tp{nm}_{m}_{k}")
                nc.tensor.transpose(tp[:], src[:, k * 128:(k + 1) * 128],
                                    identh[:])
                nc.scalar.copy(zt[m][:, k * 128:(k + 1) * 128], tp[:])

        # ---------------- b -> broadcast fp16 ----------------
        # staged in the (not yet used) g1 ring slot of m=0
        b2d = b_d.rearrange("(p n) -> p n", p=1)
        b16 = gsl[1][0]
        for h in range(NH):
            b1 = scratch.tile([1, NW], F32, tag="s", name=f"b1_{h}")
            nc.sync.dma_start(b1[:], b2d[:, h * NW:(h + 1) * NW])
            bsl = scratch.tile([128, NW], F32, tag="s", name=f"bsl{h}")
            nc.gpsimd.partition_broadcast(bsl[:], b1[:])
            nc.vector.tensor_copy(b16[:, h * NW:(h + 1) * NW], bsl[:])

        # ---------------- x load, fp16, transpose ----------------
        # x16 staged in the (not yet used) g2 ring slots
        for m in range(MT):
            for h in range(NH):
                xs = scratch.tile([128, NW], F32, tag="s", name=f"xs{m}_{h}")
                nc.sync.dma_start(xs[:], x_d[m * 128:(m + 1) * 128,
                                             h * NW:(h + 1) * NW])
                nc.vector.tensor_copy(gsl[2][m][:, h * NW:(h + 1) * NW],
                                      xs[:])
            transpose_into_zt(m, gsl[2][m][:], "x")

        # ---------------- xwx = x @ W_x + b ----------------
        for ns in range(NS):
            ps = [yps.tile([128, SW], F32, tag="yp", name=f"xwps{ns}_{m}")
                  for m in range(MT)]
            for k in range(KT):
                wxs = scratch.tile([128, SW], F32, tag="s", name=f"wxs{ns}_{k}")
                nc.sync.dma_start(wxs[:], wx_d[k * 128:(k + 1) * 128,
                                               ns * SW:(ns + 1) * SW])
                w16 = wtmp.tile([128, SW], F16, tag="w", name=f"wx16{ns}_{k}")
                nc.vector.tensor_copy(w16[:], wxs[:])
                for m in range(MT):
                    nc.tensor.matmul(
                        ps[m][:], zt[m][:, k * 128:(k + 1) * 128],
                        w16[:], start=(k == 0), stop=(k == KT - 1))
            for m in range(MT):
                sl = slice(ns * SW, (ns + 1) * SW)
                stt(xwx[m][:, sl], ps[m][:], 1.0,
                    b16[:, sl], ALU.mult, ALU.add)

        # ---------------- W_z load + fp16 convert ----------------
        for k in range(KT):
            for h in range(NH):
                wzs = scratch.tile([128, NW], F32, tag="s", name=f"wzs{k}_{h}")
                nc.sync.dma_start(wzs[:], wz_d[k * 128:(k + 1) * 128,
                                               h * NW:(h + 1) * NW])
                nc.vector.tensor_copy(
                    wz16[:, k * D + h * NW:k * D + (h + 1) * NW], wzs[:])

        # ---------------- iteration 0: z=0 ----------------
        # g0 = tanh(xwx); u0 = beta*g0; z1 aliases u0
        for m in range(MT):
            nc.scalar.activation(gsl[0][m][:], xwx[m][:], ACTF.Tanh)
            nc.vector.tensor_scalar_mul(usl[0][m][:], gsl[0][m][:], BETA)

        # z_i lives in u-ring slot zslot[i]
        zslot = [None, 0, 1, 2, 1, 2]
        hist = {}  # (kind, i, m) -> [128,1] ap

        for i in range(1, NITER):
            gi = gsl[i % 3]
            # z_i^T for all m first: these PE ops depend only on last
            # iteration's z, so the PE never stalls mid-iteration
            for m in range(MT):
                transpose_into_zt(m, usl[zslot[i]][m][:], f"i{i}")
            for m in range(MT):
                # matmul: y = z @ W_z + xwx, f = tanh(y)
                for ns in range(NS):
                    ps = yps.tile([128, SW], F32, tag="yp",
                                  name=f"yp{i}_{m}_{ns}")
                    sl = slice(ns * SW, (ns + 1) * SW)
                    nc.tensor.matmul(ps[:], identh[:], xwx[m][:, sl],
                                     start=True, stop=False)
                    for k in range(KT):
                        nc.tensor.matmul(
                            ps[:], zt[m][:, k * 128:(k + 1) * 128],
                            wz16[:, k * D + ns * SW:k * D + (ns + 1) * SW],
                            start=False, stop=(k == KT - 1))
                    nc.scalar.activation(gi[m][:, sl], ps[:], ACTF.Tanh)

                z = usl[zslot[i]][m]
                g = gi[m]
                # g = f - z ; u_i = beta*g + z (u_i slot == z slot for i>=4)
                stt(g[:], g[:], 1.0, z[:], ALU.mult, ALU.subtract)
                stt(usl[i % 3][m][:], g[:], BETA, z[:], ALU.mult, ALU.add)

                # P = <g,g> on ACT (square + accum), dump into zt[m]
                pp = dnew(f"p{i}_{m}")
                nc.scalar.activation(zt[m][:], g[:], ACTF.Square,
                                     accum_out=pp)
                hist["P", i, m] = pp

                if i >= 2:
                    q1 = dnew(f"q1_{i}_{m}")
                    nc.vector.scalar_tensor_tensor(
                        out=zt[m][:], in0=g[:], scalar=1.0,
                        in1=gsl[(i - 1) % 3][m][:],
                        op0=ALU.mult, op1=ALU.mult, accum_out=q1)
                    hist["Q1", i, m] = q1
                if i >= 3:
                    q2 = dnew(f"q2_{i}_{m}")
                    nc.vector.scalar_tensor_tensor(
                        out=zt[m][:], in0=g[:], scalar=1.0,
                        in1=gsl[(i - 2) % 3][m][:],
                        op0=ALU.mult, op1=ALU.mult, accum_out=q2)

                if i < 3:
                    # z_{i+1} = u_i (alias); transposed at next iter head
                    continue

                # ---- 2x2 regularized Anderson solve ----
                P = hist["P", i, m]
                Q1 = hist["Q1", i, m]
                Q2 = q2
                S11 = hist["P", i - 1, m]
                S12 = hist["Q1", i - 1, m]
                S22 = hist["P", i - 2, m]

                r0 = dnew(f"r0_{i}_{m}"); ts(r0, P, Q1, None, ALU.subtract)
                r1 = dnew(f"r1_{i}_{m}"); ts(r1, P, Q2, None, ALU.subtract)
                a1 = dnew(f"a1_{i}_{m}")
                ts(a1, Q1, -2.0, S11, ALU.mult, ALU.add)
                av = dnew(f"av_{i}_{m}")
                ts(av, a1, LAM, P, ALU.add, ALU.add)
                d1 = dnew(f"d1_{i}_{m}")
                ts(d1, Q2, -2.0, S22, ALU.mult, ALU.add)
                dv = dnew(f"dv_{i}_{m}")
                ts(dv, d1, LAM, P, ALU.add, ALU.add)
                b0 = dnew(f"b0_{i}_{m}"); ts(b0, Q1, Q2, None, ALU.add)
                b1t = dnew(f"b1_{i}_{m}"); ts(b1t, S12, P, None, ALU.add)
                bv = dnew(f"bv_{i}_{m}")
                ts(bv, b1t, b0, None, ALU.subtract)
                t4 = dnew(f"t4_{i}_{m}"); ts(t4, av, dv, None, ALU.mult)
                t5 = dnew(f"t5_{i}_{m}"); ts(t5, bv, bv, None, ALU.mult)
                det = dnew(f"det_{i}_{m}")
                ts(det, t4, t5, 1e-8, ALU.subtract, ALU.add)
                idet = dnew(f"idet_{i}_{m}")
                nc.vector.reciprocal(idet, det)
                g1a = dnew(f"g1a_{i}_{m}"); ts(g1a, dv, r0, None, ALU.mult)
                g1b = dnew(f"g1b_{i}_{m}"); ts(g1b, bv, r1, None, ALU.mult)
                gam1 = dnew(f"gam1_{i}_{m}")
                ts(gam1, g1a, g1b, idet, ALU.subtract, ALU.mult)
                g2a = dnew(f"g2a_{i}_{m}"); ts(g2a, av, r1, None, ALU.mult)
                g2b = dnew(f"g2b_{i}_{m}"); ts(g2b, bv, r0, None, ALU.mult)
                gam2 = dnew(f"gam2_{i}_{m}")
                ts(gam2, g2a, g2b, idet, ALU.subtract, ALU.mult)
                s0a = dnew(f"s0a_{i}_{m}"); ts(s0a, gam1, gam2, None, ALU.add)
                s0 = dnew(f"s0_{i}_{m}")
                ts(s0, s0a, -1.0, 1.0, ALU.mult, ALU.add)

                # z_{i+1} = s0*u_i + gam1*u_{i-1} + gam2*u_{i-2},
                # into u_{i-2}'s slot
                u_i = usl[i % 3][m]
                u_1 = usl[(i - 1) % 3][m]
                u_2 = usl[(i - 2) % 3][m]
                if i < NITER - 1:
                    nc.scalar.mul(u_2[:], u_2[:], gam2)
                    stt(u_2[:], u_1[:], gam1, u_2[:], ALU.mult, ALU.add)
                    stt(u_2[:], u_i[:], s0, u_2[:], ALU.mult, ALU.add)
                else:
                    for h in range(NH):
                        sl = slice(h * NW, (h + 1) * NW)
                        o32 = scratch.tile([128, NW], F32, tag="s",
                                           name=f"o32_{m}_{h}")
                        nc.scalar.activation(o32[:], u_2[:, sl],
                                             ACTF.Identity, scale=gam2)
                        stt(o32[:], u_1[:, sl], gam1, o32[:],
                            ALU.mult, ALU.add)
                        stt(o32[:], u_i[:, sl], s0, o32[:],
                            ALU.mult, ALU.add)
                        nc.sync.dma_start(
                            out_d[m * 128:(m + 1) * 128, sl], o32[:])

        if NITER < MAX_ITER:
            # debug runs: dump whatever z slot is current
            zz = usl[zslot[NITER] if NITER >= 1 else 0]
            for m in range(MT):
                for h in range(NH):
                    sl = slice(h * NW, (h + 1) * NW)
                    o32 = scratch.tile([128, NW], F32, tag="s",
                                       name=f"oz{m}_{h}")
                    nc.vector.tensor_copy(o32[:], zz[m][:, sl])
                    nc.sync.dma_start(out_d[m * 128:(m + 1) * 128, sl],
                                      o32[:])

    nc.compile()
    return nc


def kernel(x_input, W_z, W_x, b):
    x_input = np.ascontiguousarray(x_input, dtype=np.float32)
    W_z = np.ascontiguousarray(W_z, dtype=np.float32)
    W_x = np.ascontiguousarray(W_x, dtype=np.float32)
    b = np.ascontiguousarray(b, dtype=np.float32)

    if "nc" not in _CACHE:
        _CACHE["nc"] = _build()
    nc = _CACHE["nc"]

    in_maps = [{
        "x": x_input[i * BC:(i + 1) * BC],
        "W_z": W_z, "W_x": W_x, "b": b,
    } for i in range(NCORES)]

    res = bass_utils.run_bass_kernel_spmd(nc, in_maps,
                                          core_ids=list(range(NCORES)),
                                          tmpdir=os.environ.get("K_TMPDIR"))
    _CACHE["res"] = res
    out = np.concatenate([res.results[i]["z_out"] for i in range(NCORES)],
                         axis=0)
    return out.astype(np.float32)


# revision 11
# speedup vs baseline: 1.8126x; 1.0060x over previous
"""DEQ block (Anderson acceleration, 6 iters, m=3) on 8 trn2 NeuronCores.

Data-parallel over batch: each core gets 512 of 4096 samples; W_z/W_x/b
replicated.  Single pass: all 512 samples stay SBUF-resident as 4 m-tiles
of 128 in fp16.  z shares the u-ring slots (z_{i+1} = s0*u_i + g1*u_{i-1}
+ g2*u_{i-2} overwrites the retiring u slot).

Per iteration: z^T k-tiles come from DMA-xbar transposes (no PE transpose
pass), matmuls are fp16 N=1024 accumulating over 16 k-tiles in PSUM with
xwx added via an identity matmul, tanh drains PSUM->fp16 g on ACT, the
per-sample dots P/Q1/Q2 use ACT-square / DVE tensor_tensor_reduce accums,
and the 2x2 regularized Anderson solve runs as ~21 fused [128,1] DVE ops.
"""

import sys

sys.path.insert(0, "/opt/trn_rl_repo")

import os
import numpy as np
from contextlib import ExitStack

import concourse.bass as bass
import concourse.tile as tile
from concourse import bacc, mybir, masks
from concourse import bass_utils

F32 = mybir.dt.float32
F16 = mybir.dt.float16
ALU = mybir.AluOpType
ACTF = mybir.ActivationFunctionType

B, D = 4096, 2048
NCORES = 8
BC = B // NCORES          # 512 samples per core
MT = BC // 128            # 4 m-tiles
KT = D // 128             # 16 k-tiles
NH = 2                    # n-halves for DMA/convert chunks
NW = D // NH              # 1024-wide loads
NS = 4                    # n-slices for matmuls
SW = D // NS              # 512-wide matmuls (sliced operands need N<=512)
MAX_ITER = 6
BETA, LAM = 0.8, 1e-4

NITER = int(os.environ.get("K_NITER", str(MAX_ITER)))

_CACHE = {}


def _build():
    nc = bacc.Bacc("TRN2", target_bir_lowering=False, debug=False,
                   num_devices=NCORES)

    x_d = nc.dram_tensor("x", [BC, D], F32, kind="ExternalInput").ap()
    wz_d = nc.dram_tensor("W_z", [D, D], F32, kind="ExternalInput").ap()
    wx_d = nc.dram_tensor("W_x", [D, D], F32, kind="ExternalInput").ap()
    b_d = nc.dram_tensor("b", [D], F32, kind="ExternalInput").ap()
    out_d = nc.dram_tensor("z_out", [BC, D], F32, kind="ExternalOutput").ap()

    with tile.TileContext(nc) as tc, ExitStack() as ctx:
        state = ctx.enter_context(tc.tile_pool(name="state", bufs=1))

        wz16 = state.tile([128, KT * D], F16, tag="wz16", name="wz16")
        gsl = [[state.tile([128, D], F16, tag=f"g{j}_{m}", name=f"g{j}_{m}")
                for m in range(MT)] for j in range(3)]
        usl = [[state.tile([128, D], F16, tag=f"u{j}_{m}", name=f"u{j}_{m}")
                for m in range(MT)] for j in range(3)]
        xwx = [state.tile([128, D], F16, tag=f"xwx{m}", name=f"xwx{m}")
               for m in range(MT)]
        zt = [state.tile([128, D], F16, tag=f"zt{m}", name=f"zt{m}")
              for m in range(MT)]
        identh = state.tile([128, 128], F16, tag="idh", name="idh")

        scratch = ctx.enter_context(tc.tile_pool(name="scratch", bufs=2))
        wtmp = ctx.enter_context(tc.tile_pool(name="wtmp", bufs=2))
        dots = ctx.enter_context(tc.tile_pool(name="dots", bufs=64))
        yps = ctx.enter_context(tc.tile_pool(name="yps", bufs=6, space="PSUM"))
        typs = ctx.enter_context(tc.tile_pool(name="typs", bufs=2, space="PSUM"))

        id32 = scratch.tile([128, NW], F32, tag="s", name="id32")
        masks.make_identity(nc, id32[:, 0:128])
        nc.vector.tensor_copy(identh[:], id32[:, 0:128])

        def stt(out, in0, scalar, in1, op0, op1):
            nc.vector.scalar_tensor_tensor(
                out=out, in0=in0, scalar=scalar, in1=in1, op0=op0, op1=op1)

        def ts(out, in0, s1, s2, op0, op1=None):
            nc.vector.tensor_scalar(out, in0, s1, s2, op0,
                                    *([op1] if op1 is not None else []))

        def dnew(nm):
            return dots.tile([128, 1], F32, tag="d", name=nm)[:]

        def transpose_into_zt(m, src, nm):
            for k in range(KT):
                tp = typs.tile([128, 128], F16, tag="tp", name=f"tp{nm}_{m}_{k}")
                nc.tensor.transpose(tp[:], src[:, k * 128:(k + 1) * 128],
                                    identh[:])
                nc.scalar.copy(zt[m][:, k * 128:(k + 1) * 128], tp[:])

        # ---------------- b -> broadcast fp16 ----------------
        # staged in the (not yet used) g1 ring slot of m=0
        b2d = b_d.rearrange("(p n) -> p n", p=1)
        b16 = gsl[1][0]
        for h in range(NH):
            b1 = scratch.tile([1, NW], F32, tag="s", name=f"b1_{h}")
            nc.sync.dma_start(b1[:], b2d[:, h * NW:(h + 1) * NW])
            bsl = scratch.tile([128, NW], F32, tag="s", name=f"bsl{h}")
            nc.gpsimd.partition_broadcast(bsl[:], b1[:])
            nc.vector.tensor_copy(b16[:, h * NW:(h + 1) * NW], bsl[:])

        # ---------------- x load (cast-DMA fp32->fp16), transpose ----------
        # x16 staged in the (not yet used) g2 ring slots
        for m in range(MT):
            nc.gpsimd.dma_start(gsl[2][m][:], x_d[m * 128:(m + 1) * 128, :])
            transpose_into_zt(m, gsl[2][m][:], "x")

        # ---------------- xwx = x @ W_x + b ----------------
        # W_x streamed once as [128,512] fp32->fp16 cast-DMA col-chunks;
        # 4 waves of 4 live PSUM tiles
        for ns in range(NS):
            ps = [yps.tile([128, SW], F32, tag="yp", name=f"xwps{ns}_{m}")
                  for m in range(MT)]
            for k in range(KT):
                w16 = wtmp.tile([128, SW], F16, tag="w", name=f"wx16{ns}_{k}")
                nc.gpsimd.dma_start(w16[:], wx_d[k * 128:(k + 1) * 128,
                                                 ns * SW:(ns + 1) * SW])
                for m in range(MT):
                    nc.tensor.matmul(
                        ps[m][:], zt[m][:, k * 128:(k + 1) * 128],
                        w16[:], start=(k == 0), stop=(k == KT - 1))
            for m in range(MT):
                sl = slice(ns * SW, (ns + 1) * SW)
                stt(xwx[m][:, sl], ps[m][:], 1.0,
                    b16[:, sl], ALU.mult, ALU.add)

        # ---------------- W_z load (cast-DMA fp32->fp16) ----------------
        for k in range(KT):
            nc.gpsimd.dma_start(wz16[:, k * D:(k + 1) * D],
                                wz_d[k * 128:(k + 1) * 128, :])

        # ---------------- iteration 0: z=0 ----------------
        # g0 = tanh(xwx); u0 = beta*g0; z1 aliases u0
        for m in range(MT):
            nc.scalar.activation(gsl[0][m][:], xwx[m][:], ACTF.Tanh)
            nc.vector.tensor_scalar_mul(usl[0][m][:], gsl[0][m][:], BETA)

        # z_i lives in u-ring slot zslot[i]
        zslot = [None, 0, 1, 2, 1, 2]
        hist = {}  # (kind, i, m) -> [128,1] ap

        for i in range(1, NITER):
            gi = gsl[i % 3]
            # z_i^T for all m first: these PE ops depend only on last
            # iteration's z, so the PE never stalls mid-iteration
            for m in range(MT):
                transpose_into_zt(m, usl[zslot[i]][m][:], f"i{i}")
            for m in range(MT):
                # matmul: y = z @ W_z + xwx, f = tanh(y)
                for ns in range(NS):
                    ps = yps.tile([128, SW], F32, tag="yp",
                                  name=f"yp{i}_{m}_{ns}")
                    sl = slice(ns * SW, (ns + 1) * SW)
                    nc.tensor.matmul(ps[:], identh[:], xwx[m][:, sl],
                                     start=True, stop=False)
                    for k in range(KT):
                        nc.tensor.matmul(
                            ps[:], zt[m][:, k * 128:(k + 1) * 128],
                            wz16[:, k * D + ns * SW:k * D + (ns + 1) * SW],
                            start=False, stop=(k == KT - 1))
                    nc.scalar.activation(gi[m][:, sl], ps[:], ACTF.Tanh)

                z = usl[zslot[i]][m]
                g = gi[m]
                # g = f - z ; u_i = beta*g + z (u_i slot == z slot for i>=4)
                stt(g[:], g[:], 1.0, z[:], ALU.mult, ALU.subtract)
                stt(usl[i % 3][m][:], g[:], BETA, z[:], ALU.mult, ALU.add)

                # P = <g,g> on ACT (square + accum), dump into zt[m]
                pp = dnew(f"p{i}_{m}")
                nc.scalar.activation(zt[m][:], g[:], ACTF.Square,
                                     accum_out=pp)
                hist["P", i, m] = pp

                if i >= 2:
                    q1 = dnew(f"q1_{i}_{m}")
                    nc.vector.scalar_tensor_tensor(
                        out=zt[m][:], in0=g[:], scalar=1.0,
                        in1=gsl[(i - 1) % 3][m][:],
                        op0=ALU.mult, op1=ALU.mult, accum_out=q1)
                    hist["Q1", i, m] = q1
                if i >= 3:
                    q2 = dnew(f"q2_{i}_{m}")
                    nc.vector.scalar_tensor_tensor(
                        out=zt[m][:], in0=g[:], scalar=1.0,
                        in1=gsl[(i - 2) % 3][m][:],
                        op0=ALU.mult, op1=ALU.mult, accum_out=q2)

                if i < 3:
                    # z_{i+1} = u_i (alias); transposed at next iter head
                    continue

                # ---- 2x2 regularized Anderson solve ----
                P = hist["P", i, m]
                Q1 = hist["Q1", i, m]
                Q2 = q2
                S11 = hist["P", i - 1, m]
                S12 = hist["Q1", i - 1, m]
                S22 = hist["P", i - 2, m]

                r0 = dnew(f"r0_{i}_{m}"); ts(r0, P, Q1, None, ALU.subtract)
                r1 = dnew(f"r1_{i}_{m}"); ts(r1, P, Q2, None, ALU.subtract)
                a1 = dnew(f"a1_{i}_{m}")
                ts(a1, Q1, -2.0, S11, ALU.mult, ALU.add)
                av = dnew(f"av_{i}_{m}")
                ts(av, a1, LAM, P, ALU.add, ALU.add)
                d1 = dnew(f"d1_{i}_{m}")
                ts(d1, Q2, -2.0, S22, ALU.mult, ALU.add)
                dv = dnew(f"dv_{i}_{m}")
                ts(dv, d1, LAM, P, ALU.add, ALU.add)
                b0 = dnew(f"b0_{i}_{m}"); ts(b0, Q1, Q2, None, ALU.add)
                b1t = dnew(f"b1_{i}_{m}"); ts(b1t, S12, P, None, ALU.add)
                bv = dnew(f"bv_{i}_{m}")
                ts(bv, b1t, b0, None, ALU.subtract)
                t4 = dnew(f"t4_{i}_{m}"); ts(t4, av, dv, None, ALU.mult)
                t5 = dnew(f"t5_{i}_{m}"); ts(t5, bv, bv, None, ALU.mult)
                det = dnew(f"det_{i}_{m}")
                ts(det, t4, t5, 1e-8, ALU.subtract, ALU.add)
                idet = dnew(f"idet_{i}_{m}")
                nc.vector.reciprocal(idet, det)
                g1a = dnew(f"g1a_{i}_{m}"); ts(g1a, dv, r0, None, ALU.mult)
                g1b = dnew(f"g1b_{i}_{m}"); ts(g1b, bv, r1, None, ALU.mult)
                gam1 = dnew(f"gam1_{i}_{m}")
                ts(gam1, g1a, g1b, idet, ALU.subtract, ALU.mult)
                g2a = dnew(f"g2a_{i}_{m}"); ts(g2a, av, r1, None, ALU.mult)
                g2b = dnew(f"g2b_{i}_{m}"); ts(g2b, bv, r0, None, ALU.mult)
                gam2 = dnew(f"gam2_{i}_{m}")
                ts(gam2, g2a, g2b, idet, ALU.subtract, ALU.mult)
                s0a = dnew(f"s0a_{i}_{m}"); ts(s0a, gam1, gam2, None, ALU.add)
                s0 = dnew(f"s0_{i}_{m}")
                ts(s0, s0a, -1.0, 1.0, ALU.mult, ALU.add)

                # z_{i+1} = s0*u_i + gam1*u_{i-1} + gam2*u_{i-2},
                # into u_{i-2}'s slot
                u_i = usl[i % 3][m]
                u_1 = usl[(i - 1) % 3][m]
                u_2 = usl[(i - 2) % 3][m]
                if i < NITER - 1:
                    nc.scalar.mul(u_2[:], u_2[:], gam2)
                    stt(u_2[:], u_1[:], gam1, u_2[:], ALU.mult, ALU.add)
                    stt(u_2[:], u_i[:], s0, u_2[:], ALU.mult, ALU.add)
                else:
                    for h in range(NH):
                        sl = slice(h * NW, (h + 1) * NW)
                        o32 = scratch.tile([128, NW], F32, tag="s",
                                           name=f"o32_{m}_{h}")
                        nc.scalar.activation(o32[:], u_2[:, sl],
                                             ACTF.Identity, scale=gam2)
                        stt(o32[:], u_1[:, sl], gam1, o32[:],
                            ALU.mult, ALU.add)
                        stt(o32[:], u_i[:, sl], s0, o32[:],
                            ALU.mult, ALU.add)
                        nc.sync.dma_start(
                            out_d[m * 128:(m + 1) * 128, sl], o32[:])

        if NITER < MAX_ITER:
            # debug runs: dump whatever z slot is current
            zz = usl[zslot[NITER] if NITER >= 1 else 0]
            for m in range(MT):
                for h in range(NH):
                    sl = slice(h * NW, (h + 1) * NW)
                    o32 = scratch.tile([128, NW], F32, tag="s",
                                       name=f"oz{m}_{h}")
                    nc.vector.tensor_copy(o32[:], zz[m][:, sl])
                    nc.sync.dma_start(out_d[m * 128:(m + 1) * 128, sl],
                                      o32[:])

    nc.compile()
    return nc


def kernel(x_input, W_z, W_x, b):
    x_input = np.ascontiguousarray(x_input, dtype=np.float32)
    W_z = np.ascontiguousarray(W_z, dtype=np.float32)
    W_x = np.ascontiguousarray(W_x, dtype=np.float32)
    b = np.ascontiguousarray(b, dtype=np.float32)

    if "nc" not in _CACHE:
        _CACHE["nc"] = _build()
    nc = _CACHE["nc"]

    in_maps = [{
        "x": x_input[i * BC:(i + 1) * BC],
        "W_z": W_z, "W_x": W_x, "b": b,
    } for i in range(NCORES)]

    res = bass_utils.run_bass_kernel_spmd(nc, in_maps,
                                          core_ids=list(range(NCORES)),
                                          tmpdir=os.environ.get("K_TMPDIR"))
    _CACHE["res"] = res
    out = np.concatenate([res.results[i]["z_out"] for i in range(NCORES)],
                         axis=0)
    return out.astype(np.float32)


# revision 13
# speedup vs baseline: 2.3706x; 1.3078x over previous
"""DEQ block (Anderson acceleration, 6 iters, m=3) on 8 trn2 NeuronCores.

Data-parallel over batch: each core gets 512 of 4096 samples; W_z/W_x/b
replicated.  Single pass: all 512 samples stay SBUF-resident as 4 m-tiles
of 128 in fp16.  z shares the u-ring slots (z_{i+1} = s0*u_i + g1*u_{i-1}
+ g2*u_{i-2} overwrites the retiring u slot).

Per iteration: z^T k-tiles come from DMA-xbar transposes (no PE transpose
pass), matmuls are fp16 N=1024 accumulating over 16 k-tiles in PSUM with
xwx added via an identity matmul, tanh drains PSUM->fp16 g on ACT, the
per-sample dots P/Q1/Q2 use ACT-square / DVE tensor_tensor_reduce accums,
and the 2x2 regularized Anderson solve runs as ~21 fused [128,1] DVE ops.
"""

import sys

sys.path.insert(0, "/opt/trn_rl_repo")

import os
import numpy as np
from contextlib import ExitStack

import concourse.bass as bass
import concourse.tile as tile
from concourse import bacc, mybir, masks
from concourse import bass_utils

F32 = mybir.dt.float32
F16 = mybir.dt.float16
ALU = mybir.AluOpType
ACTF = mybir.ActivationFunctionType

B, D = 4096, 2048
NCORES = 8
BC = B // NCORES          # 512 samples per core
MT = BC // 128            # 4 m-tiles
KT = D // 128             # 16 k-tiles
NH = 2                    # n-halves for DMA/convert chunks
NW = D // NH              # 1024-wide loads
NS = 4                    # n-slices for matmuls
SW = D // NS              # 512-wide matmuls (sliced operands need N<=512)
MAX_ITER = 6
BETA, LAM = 0.8, 1e-4

NITER = int(os.environ.get("K_NITER", str(MAX_ITER)))

_CACHE = {}


def _build():
    nc = bacc.Bacc("TRN2", target_bir_lowering=False, debug=False,
                   num_devices=NCORES)

    x_d = nc.dram_tensor("x", [BC, D], F16, kind="ExternalInput").ap()
    wz_d = nc.dram_tensor("W_z", [D, D], F16, kind="ExternalInput").ap()
    wx_d = nc.dram_tensor("W_x", [D, D], F16, kind="ExternalInput").ap()
    b_d = nc.dram_tensor("b", [D], F32, kind="ExternalInput").ap()
    out_d = nc.dram_tensor("z_out", [BC, D], F32, kind="ExternalOutput").ap()

    with tile.TileContext(nc) as tc, ExitStack() as ctx:
        state = ctx.enter_context(tc.tile_pool(name="state", bufs=1))

        wz16 = state.tile([128, KT * D], F16, tag="wz16", name="wz16")
        gsl = [[state.tile([128, D], F16, tag=f"g{j}_{m}", name=f"g{j}_{m}")
                for m in range(MT)] for j in range(3)]
        usl = [[state.tile([128, D], F16, tag=f"u{j}_{m}", name=f"u{j}_{m}")
                for m in range(MT)] for j in range(3)]
        xwx = [state.tile([128, D], F16, tag=f"xwx{m}", name=f"xwx{m}")
               for m in range(MT)]
        zt = [state.tile([128, D], F16, tag=f"zt{m}", name=f"zt{m}")
              for m in range(MT)]
        identh = state.tile([128, 128], F16, tag="idh", name="idh")

        scratch = ctx.enter_context(tc.tile_pool(name="scratch", bufs=2))
        wtmp = ctx.enter_context(tc.tile_pool(name="wtmp", bufs=5))
        dots = ctx.enter_context(tc.tile_pool(name="dots", bufs=64))
        yps = ctx.enter_context(tc.tile_pool(name="yps", bufs=6, space="PSUM"))
        typs = ctx.enter_context(tc.tile_pool(name="typs", bufs=2, space="PSUM"))

        id32 = scratch.tile([128, NW], F32, tag="s", name="id32")
        masks.make_identity(nc, id32[:, 0:128])
        nc.vector.tensor_copy(identh[:], id32[:, 0:128])

        def stt(out, in0, scalar, in1, op0, op1):
            nc.vector.scalar_tensor_tensor(
                out=out, in0=in0, scalar=scalar, in1=in1, op0=op0, op1=op1)

        def ts(out, in0, s1, s2, op0, op1=None):
            nc.vector.tensor_scalar(out, in0, s1, s2, op0,
                                    *([op1] if op1 is not None else []))

        def dnew(nm):
            return dots.tile([128, 1], F32, tag="d", name=nm)[:]

        def transpose_into_zt(m, src, nm):
            for k in range(KT):
                tp = typs.tile([128, 128], F16, tag="tp", name=f"tp{nm}_{m}_{k}")
                nc.tensor.transpose(tp[:], src[:, k * 128:(k + 1) * 128],
                                    identh[:])
                nc.scalar.copy(zt[m][:, k * 128:(k + 1) * 128], tp[:])

        # ---------------- b -> broadcast fp16 ----------------
        # staged in the (not yet used) g1 ring slot of m=0
        b2d = b_d.rearrange("(p n) -> p n", p=1)
        b16 = gsl[1][0]
        for h in range(NH):
            b1 = scratch.tile([1, NW], F32, tag="s", name=f"b1_{h}")
            nc.sync.dma_start(b1[:], b2d[:, h * NW:(h + 1) * NW])
            bsl = scratch.tile([128, NW], F32, tag="s", name=f"bsl{h}")
            nc.gpsimd.partition_broadcast(bsl[:], b1[:])
            nc.vector.tensor_copy(b16[:, h * NW:(h + 1) * NW], bsl[:])

        # ---------------- x load (cast-DMA fp32->fp16), transpose ----------
        # x16 staged in the (not yet used) g2 ring slots
        for m in range(MT):
            nc.sync.dma_start(gsl[2][m][:], x_d[m * 128:(m + 1) * 128, :])
            transpose_into_zt(m, gsl[2][m][:], "x")

        # ---------------- xwx = x @ W_x + b ----------------
        # W_x streamed once as [128,512] fp32->fp16 cast-DMA col-chunks;
        # 4 waves of 4 live PSUM tiles
        for ns in range(NS):
            ps = [yps.tile([128, SW], F32, tag="yp", name=f"xwps{ns}_{m}")
                  for m in range(MT)]
            for k in range(KT):
                w16 = wtmp.tile([128, SW], F16, tag="w", name=f"wx16{ns}_{k}")
                nc.sync.dma_start(w16[:], wx_d[k * 128:(k + 1) * 128,
                                               ns * SW:(ns + 1) * SW])
                for m in range(MT):
                    nc.tensor.matmul(
                        ps[m][:], zt[m][:, k * 128:(k + 1) * 128],
                        w16[:], start=(k == 0), stop=(k == KT - 1))
            for m in range(MT):
                sl = slice(ns * SW, (ns + 1) * SW)
                stt(xwx[m][:, sl], ps[m][:], 1.0,
                    b16[:, sl], ALU.mult, ALU.add)

        # ---------------- W_z load (fp16 in DRAM) ----------------
        for k in range(KT):
            nc.sync.dma_start(wz16[:, k * D:(k + 1) * D],
                              wz_d[k * 128:(k + 1) * 128, :])

        # ---------------- iteration 0: z=0 ----------------
        # g0 = tanh(xwx); u0 = beta*g0; z1 aliases u0
        for m in range(MT):
            nc.scalar.activation(gsl[0][m][:], xwx[m][:], ACTF.Tanh)
            nc.vector.tensor_scalar_mul(usl[0][m][:], gsl[0][m][:], BETA)

        # z_i lives in u-ring slot zslot[i]
        zslot = [None, 0, 1, 2, 1, 2]
        hist = {}  # (kind, i, m) -> [128,1] ap

        for i in range(1, NITER):
            gi = gsl[i % 3]
            # z_i^T for all m first: these PE ops depend only on last
            # iteration's z, so the PE never stalls mid-iteration
            for m in range(MT):
                transpose_into_zt(m, usl[zslot[i]][m][:], f"i{i}")
            for m in range(MT):
                # matmul: y = z @ W_z + xwx, f = tanh(y)
                for ns in range(NS):
                    ps = yps.tile([128, SW], F32, tag="yp",
                                  name=f"yp{i}_{m}_{ns}")
                    sl = slice(ns * SW, (ns + 1) * SW)
                    nc.tensor.matmul(ps[:], identh[:], xwx[m][:, sl],
                                     start=True, stop=False)
                    for k in range(KT):
                        nc.tensor.matmul(
                            ps[:], zt[m][:, k * 128:(k + 1) * 128],
                            wz16[:, k * D + ns * SW:k * D + (ns + 1) * SW],
                            start=False, stop=(k == KT - 1))
                    nc.scalar.activation(gi[m][:, sl], ps[:], ACTF.Tanh)

                z = usl[zslot[i]][m]
                g = gi[m]
                # g = f - z ; u_i = beta*g + z (u_i slot == z slot for i>=4)
                stt(g[:], g[:], 1.0, z[:], ALU.mult, ALU.subtract)
                stt(usl[i % 3][m][:], g[:], BETA, z[:], ALU.mult, ALU.add)

                # P = <g,g> on ACT (square + accum), dump into zt[m]
                pp = dnew(f"p{i}_{m}")
                nc.scalar.activation(zt[m][:], g[:], ACTF.Square,
                                     accum_out=pp)
                hist["P", i, m] = pp

                if i >= 2:
                    q1 = dnew(f"q1_{i}_{m}")
                    nc.vector.scalar_tensor_tensor(
                        out=zt[m][:], in0=g[:], scalar=1.0,
                        in1=gsl[(i - 1) % 3][m][:],
                        op0=ALU.mult, op1=ALU.mult, accum_out=q1)
                    hist["Q1", i, m] = q1
                if i >= 3:
                    q2 = dnew(f"q2_{i}_{m}")
                    nc.vector.scalar_tensor_tensor(
                        out=zt[m][:], in0=g[:], scalar=1.0,
                        in1=gsl[(i - 2) % 3][m][:],
                        op0=ALU.mult, op1=ALU.mult, accum_out=q2)

                if i < 3:
                    # z_{i+1} = u_i (alias); transposed at next iter head
                    continue

                # ---- 2x2 regularized Anderson solve ----
                P = hist["P", i, m]
                Q1 = hist["Q1", i, m]
                Q2 = q2
                S11 = hist["P", i - 1, m]
                S12 = hist["Q1", i - 1, m]
                S22 = hist["P", i - 2, m]

                r0 = dnew(f"r0_{i}_{m}"); ts(r0, P, Q1, None, ALU.subtract)
                r1 = dnew(f"r1_{i}_{m}"); ts(r1, P, Q2, None, ALU.subtract)
                a1 = dnew(f"a1_{i}_{m}")
                ts(a1, Q1, -2.0, S11, ALU.mult, ALU.add)
                av = dnew(f"av_{i}_{m}")
                ts(av, a1, LAM, P, ALU.add, ALU.add)
                d1 = dnew(f"d1_{i}_{m}")
                ts(d1, Q2, -2.0, S22, ALU.mult, ALU.add)
                dv = dnew(f"dv_{i}_{m}")
                ts(dv, d1, LAM, P, ALU.add, ALU.add)
                b0 = dnew(f"b0_{i}_{m}"); ts(b0, Q1, Q2, None, ALU.add)
                b1t = dnew(f"b1_{i}_{m}"); ts(b1t, S12, P, None, ALU.add)
                bv = dnew(f"bv_{i}_{m}")
                ts(bv, b1t, b0, None, ALU.subtract)
                t4 = dnew(f"t4_{i}_{m}"); ts(t4, av, dv, None, ALU.mult)
                t5 = dnew(f"t5_{i}_{m}"); ts(t5, bv, bv, None, ALU.mult)
                det = dnew(f"det_{i}_{m}")
                ts(det, t4, t5, 1e-8, ALU.subtract, ALU.add)
                idet = dnew(f"idet_{i}_{m}")
                nc.vector.reciprocal(idet, det)
                g1a = dnew(f"g1a_{i}_{m}"); ts(g1a, dv, r0, None, ALU.mult)
                g1b = dnew(f"g1b_{i}_{m}"); ts(g1b, bv, r1, None, ALU.mult)
                gam1 = dnew(f"gam1_{i}_{m}")
                ts(gam1, g1a, g1b, idet, ALU.subtract, ALU.mult)
                g2a = dnew(f"g2a_{i}_{m}"); ts(g2a, av, r1, None, ALU.mult)
                g2b = dnew(f"g2b_{i}_{m}"); ts(g2b, bv, r0, None, ALU.mult)
                gam2 = dnew(f"gam2_{i}_{m}")
                ts(gam2, g2a, g2b, idet, ALU.subtract, ALU.mult)
                s0a = dnew(f"s0a_{i}_{m}"); ts(s0a, gam1, gam2, None, ALU.add)
                s0 = dnew(f"s0_{i}_{m}")
                ts(s0, s0a, -1.0, 1.0, ALU.mult, ALU.add)

                # z_{i+1} = s0*u_i + gam1*u_{i-1} + gam2*u_{i-2},
                # into u_{i-2}'s slot
                u_i = usl[i % 3][m]
                u_1 = usl[(i - 1) % 3][m]
                u_2 = usl[(i - 2) % 3][m]
                if i < NITER - 1:
                    nc.scalar.mul(u_2[:], u_2[:], gam2)
                    stt(u_2[:], u_1[:], gam1, u_2[:], ALU.mult, ALU.add)
                    stt(u_2[:], u_i[:], s0, u_2[:], ALU.mult, ALU.add)
                else:
                    for h in range(NH):
                        sl = slice(h * NW, (h + 1) * NW)
                        o32 = scratch.tile([128, NW], F32, tag="s",
                                           name=f"o32_{m}_{h}")
                        nc.scalar.activation(o32[:], u_2[:, sl],
                                             ACTF.Identity, scale=gam2)
                        stt(o32[:], u_1[:, sl], gam1, o32[:],
                            ALU.mult, ALU.add)
                        stt(o32[:], u_i[:, sl], s0, o32[:],
                            ALU.mult, ALU.add)
                        nc.sync.dma_start(
                            out_d[m * 128:(m + 1) * 128, sl], o32[:])

        if NITER < MAX_ITER:
            # debug runs: dump whatever z slot is current
            zz = usl[zslot[NITER] if NITER >= 1 else 0]
            for m in range(MT):
                for h in range(NH):
                    sl = slice(h * NW, (h + 1) * NW)
                    o32 = scratch.tile([128, NW], F32, tag="s",
                                       name=f"oz{m}_{h}")
                    nc.vector.tensor_copy(o32[:], zz[m][:, sl])
                    nc.sync.dma_start(out_d[m * 128:(m + 1) * 128, sl],
                                      o32[:])

    nc.compile()
    return nc


def kernel(x_input, W_z, W_x, b):
    x_input = np.ascontiguousarray(x_input, dtype=np.float16)
    W_z = np.ascontiguousarray(W_z, dtype=np.float16)
    W_x = np.ascontiguousarray(W_x, dtype=np.float16)
    b = np.ascontiguousarray(b, dtype=np.float32)

    if "nc" not in _CACHE:
        _CACHE["nc"] = _build()
    nc = _CACHE["nc"]

    in_maps = [{
        "x": x_input[i * BC:(i + 1) * BC],
        "W_z": W_z, "W_x": W_x, "b": b,
    } for i in range(NCORES)]

    res = bass_utils.run_bass_kernel_spmd(nc, in_maps,
                                          core_ids=list(range(NCORES)),
                                          tmpdir=os.environ.get("K_TMPDIR"))
    _CACHE["res"] = res
    out = np.concatenate([res.results[i]["z_out"] for i in range(NCORES)],
                         axis=0)
    return out.astype(np.float32)


# revision 14
# speedup vs baseline: 2.4935x; 1.0519x over previous
"""DEQ block (Anderson acceleration, 6 iters, m=3) on 8 trn2 NeuronCores.

Data-parallel over batch: each core gets 512 of 4096 samples; W_z/W_x/b
replicated.  Single pass: all 512 samples stay SBUF-resident as 4 m-tiles
of 128 in fp16.  z shares the u-ring slots (z_{i+1} = s0*u_i + g1*u_{i-1}
+ g2*u_{i-2} overwrites the retiring u slot).

Per iteration: z^T k-tiles come from DMA-xbar transposes (no PE transpose
pass), matmuls are fp16 N=1024 accumulating over 16 k-tiles in PSUM with
xwx added via an identity matmul, tanh drains PSUM->fp16 g on ACT, the
per-sample dots P/Q1/Q2 use ACT-square / DVE tensor_tensor_reduce accums,
and the 2x2 regularized Anderson solve runs as ~21 fused [128,1] DVE ops.
"""

import sys

sys.path.insert(0, "/opt/trn_rl_repo")

import os
import numpy as np
from contextlib import ExitStack

import concourse.bass as bass
import concourse.tile as tile
from concourse import bacc, mybir, masks
from concourse import bass_utils

F32 = mybir.dt.float32
F16 = mybir.dt.float16
ALU = mybir.AluOpType
ACTF = mybir.ActivationFunctionType

B, D = 4096, 2048
NCORES = 8
BC = B // NCORES          # 512 samples per core
MT = BC // 128            # 4 m-tiles
KT = D // 128             # 16 k-tiles
NH = 2                    # n-halves for DMA/convert chunks
NW = D // NH              # 1024-wide loads
NS = 4                    # n-slices for matmuls
SW = D // NS              # 512-wide matmuls (sliced operands need N<=512)
MAX_ITER = 6
BETA, LAM = 0.8, 1e-4

NITER = int(os.environ.get("K_NITER", str(MAX_ITER)))

_CACHE = {}


def _build():
    nc = bacc.Bacc("TRN2", target_bir_lowering=False, debug=False,
                   num_devices=NCORES)

    x_d = nc.dram_tensor("x", [BC, D], F16, kind="ExternalInput").ap()
    wz_d = nc.dram_tensor("W_z", [D, D], F16, kind="ExternalInput").ap()
    wx_d = nc.dram_tensor("W_x", [D, D], F16, kind="ExternalInput").ap()
    b_d = nc.dram_tensor("b", [D], F32, kind="ExternalInput").ap()
    out_d = nc.dram_tensor("z_out", [BC, D], F32, kind="ExternalOutput").ap()

    with tile.TileContext(nc) as tc, ExitStack() as ctx:
        state = ctx.enter_context(tc.tile_pool(name="state", bufs=1))

        wz16 = state.tile([128, KT * D], F16, tag="wz16", name="wz16")
        gsl = [[state.tile([128, D], F16, tag=f"g{j}_{m}", name=f"g{j}_{m}")
                for m in range(MT)] for j in range(3)]
        usl = [[state.tile([128, D], F16, tag=f"u{j}_{m}", name=f"u{j}_{m}")
                for m in range(MT)] for j in range(3)]
        xwx = [state.tile([128, D], F16, tag=f"xwx{m}", name=f"xwx{m}")
               for m in range(MT)]
        zt = [state.tile([128, D], F16, tag=f"zt{m}", name=f"zt{m}")
              for m in range(MT)]
        identh = state.tile([128, 128], F16, tag="idh", name="idh")

        scratch = ctx.enter_context(tc.tile_pool(name="scratch", bufs=2))
        wtmp = ctx.enter_context(tc.tile_pool(name="wtmp", bufs=5))
        dots = ctx.enter_context(tc.tile_pool(name="dots", bufs=64))
        yps = ctx.enter_context(tc.tile_pool(name="yps", bufs=6, space="PSUM"))
        typs = ctx.enter_context(tc.tile_pool(name="typs", bufs=2, space="PSUM"))

        id32 = scratch.tile([128, NW], F32, tag="s", name="id32")
        masks.make_identity(nc, id32[:, 0:128])
        nc.vector.tensor_copy(identh[:], id32[:, 0:128])

        def stt(out, in0, scalar, in1, op0, op1):
            nc.vector.scalar_tensor_tensor(
                out=out, in0=in0, scalar=scalar, in1=in1, op0=op0, op1=op1)

        def ts(out, in0, s1, s2, op0, op1=None):
            nc.vector.tensor_scalar(out, in0, s1, s2, op0,
                                    *([op1] if op1 is not None else []))

        def dnew(nm):
            return dots.tile([128, 1], F32, tag="d", name=nm)[:]

        def transpose_into_zt(m, src, nm):
            for k in range(KT):
                tp = typs.tile([128, 128], F16, tag="tp", name=f"tp{nm}_{m}_{k}")
                nc.tensor.transpose(tp[:], src[:, k * 128:(k + 1) * 128],
                                    identh[:])
                nc.scalar.copy(zt[m][:, k * 128:(k + 1) * 128], tp[:])

        # ---------------- b -> broadcast fp16 ----------------
        # staged in the (not yet used) g1 ring slot of m=0
        b2d = b_d.rearrange("(p n) -> p n", p=1)
        b16 = gsl[1][0]
        for h in range(NH):
            b1 = scratch.tile([1, NW], F32, tag="s", name=f"b1_{h}")
            nc.sync.dma_start(b1[:], b2d[:, h * NW:(h + 1) * NW])
            bsl = scratch.tile([128, NW], F32, tag="s", name=f"bsl{h}")
            nc.gpsimd.partition_broadcast(bsl[:], b1[:])
            nc.vector.tensor_copy(b16[:, h * NW:(h + 1) * NW], bsl[:])

        # ---------------- x load (cast-DMA fp32->fp16), transpose ----------
        # x16 staged in the (not yet used) g2 ring slots
        for m in range(MT):
            nc.sync.dma_start(gsl[2][m][:], x_d[m * 128:(m + 1) * 128, :])
            transpose_into_zt(m, gsl[2][m][:], "x")

        # ---------------- xwx = x @ W_x + b ----------------
        # W_x streamed once as [128,512] fp32->fp16 cast-DMA col-chunks;
        # 4 waves of 4 live PSUM tiles
        for ns in range(NS):
            ps = [yps.tile([128, SW], F32, tag="yp", name=f"xwps{ns}_{m}")
                  for m in range(MT)]
            for k in range(KT):
                w16 = wtmp.tile([128, SW], F16, tag="w", name=f"wx16{ns}_{k}")
                nc.sync.dma_start(w16[:], wx_d[k * 128:(k + 1) * 128,
                                               ns * SW:(ns + 1) * SW])
                for m in range(MT):
                    nc.tensor.matmul(
                        ps[m][:], zt[m][:, k * 128:(k + 1) * 128],
                        w16[:], start=(k == 0), stop=(k == KT - 1))
            for m in range(MT):
                sl = slice(ns * SW, (ns + 1) * SW)
                stt(xwx[m][:, sl], ps[m][:], 1.0,
                    b16[:, sl], ALU.mult, ALU.add)

        # ---------------- W_z load (fp16 in DRAM) ----------------
        for k in range(KT):
            nc.sync.dma_start(wz16[:, k * D:(k + 1) * D],
                              wz_d[k * 128:(k + 1) * 128, :])

        # ---------------- iteration 0: z=0 ----------------
        # g0 = tanh(xwx); u0 = beta*g0; z1 aliases u0
        for m in range(MT):
            nc.scalar.activation(gsl[0][m][:], xwx[m][:], ACTF.Tanh)
            nc.vector.tensor_scalar_mul(usl[0][m][:], gsl[0][m][:], BETA)

        # z_i lives in u-ring slot zslot[i]
        zslot = [None, 0, 1, 2, 1, 2]
        hist = {}  # (kind, i, pair) -> [128,2] ap

        def pnew(nm):
            return dots.tile([128, 2], F32, tag="d", name=nm)[:]

        for i in range(1, NITER):
            gi = gsl[i % 3]
            for p in range(MT // 2):
                mm0 = 2 * p
                pair = (mm0, mm0 + 1)
                # z_i^T for this pair (PE): depends only on last iteration's
                # pair-p update, which finished during other MM work
                for m in pair:
                    transpose_into_zt(m, usl[zslot[i]][m][:], f"i{i}")

                pp = pnew(f"p{i}_{p}")
                q1 = pnew(f"q1_{i}_{p}") if i >= 2 else None
                q2 = pnew(f"q2_{i}_{p}") if i >= 3 else None

                for m in pair:
                    c = m - mm0
                    # matmul: y = z @ W_z + xwx, f = tanh(y)
                    for ns in range(NS):
                        ps = yps.tile([128, SW], F32, tag="yp",
                                      name=f"yp{i}_{m}_{ns}")
                        sl = slice(ns * SW, (ns + 1) * SW)
                        nc.tensor.matmul(ps[:], identh[:], xwx[m][:, sl],
                                         start=True, stop=False)
                        for k in range(KT):
                            nc.tensor.matmul(
                                ps[:], zt[m][:, k * 128:(k + 1) * 128],
                                wz16[:, k * D + ns * SW:k * D + (ns + 1) * SW],
                                start=False, stop=(k == KT - 1))
                        nc.scalar.activation(gi[m][:, sl], ps[:], ACTF.Tanh)

                    z = usl[zslot[i]][m]
                    g = gi[m]
                    # g = f - z ; u_i = beta*g + z (in-place over z for i>=4)
                    stt(g[:], g[:], 1.0, z[:], ALU.mult, ALU.subtract)
                    stt(usl[i % 3][m][:], g[:], BETA, z[:], ALU.mult, ALU.add)

                    # P = <g,g> on ACT (square + accum), dump into zt[m]
                    nc.scalar.activation(zt[m][:], g[:], ACTF.Square,
                                         accum_out=pp[:, c:c + 1])
                    if i >= 2:
                        nc.vector.scalar_tensor_tensor(
                            out=zt[m][:], in0=g[:], scalar=1.0,
                            in1=gsl[(i - 1) % 3][m][:],
                            op0=ALU.mult, op1=ALU.mult,
                            accum_out=q1[:, c:c + 1])
                    if i >= 3:
                        nc.vector.scalar_tensor_tensor(
                            out=zt[m][:], in0=g[:], scalar=1.0,
                            in1=gsl[(i - 2) % 3][m][:],
                            op0=ALU.mult, op1=ALU.mult,
                            accum_out=q2[:, c:c + 1])

                hist["P", i, p] = pp
                if i >= 2:
                    hist["Q1", i, p] = q1
                if i < 3:
                    continue

                # ---- 2x2 regularized Anderson solve, both m at once ----
                P = pp
                Q1 = hist["Q1", i, p]
                Q2 = q2
                S11 = hist["P", i - 1, p]
                S12 = hist["Q1", i - 1, p]
                S22 = hist["P", i - 2, p]

                def sv(nm, in0, scalar, in1, op0, op1):
                    o = pnew(f"{nm}_{i}_{p}")
                    stt(o, in0, scalar, in1, op0, op1)
                    return o

                r0 = sv("r0", Q1, -1.0, P, ALU.mult, ALU.add)
                r1 = sv("r1", Q2, -1.0, P, ALU.mult, ALU.add)
                a1 = sv("a1", Q1, -2.0, S11, ALU.mult, ALU.add)
                av = sv("av", a1, LAM, P, ALU.add, ALU.add)
                d1 = sv("d1", Q2, -2.0, S22, ALU.mult, ALU.add)
                dv = sv("dv", d1, LAM, P, ALU.add, ALU.add)
                b0 = sv("b0", Q1, 1.0, Q2, ALU.mult, ALU.add)
                b1t = sv("b1", S12, 1.0, P, ALU.mult, ALU.add)
                bv = sv("bv", b0, -1.0, b1t, ALU.mult, ALU.add)
                t4 = sv("t4", av, 1.0, dv, ALU.mult, ALU.mult)
                t5 = sv("t5", bv, 1.0, bv, ALU.mult, ALU.mult)
                d0 = sv("d0", t5, -1.0, t4, ALU.mult, ALU.add)
                det = pnew(f"det_{i}_{p}")
                ts(det, d0, 1e-8, None, ALU.add)
                idet = pnew(f"idet_{i}_{p}")
                nc.vector.reciprocal(idet, det)
                g1a = sv("g1a", dv, 1.0, r0, ALU.mult, ALU.mult)
                g1b = sv("g1b", bv, 1.0, r1, ALU.mult, ALU.mult)
                g1n = sv("g1n", g1b, -1.0, g1a, ALU.mult, ALU.add)
                gam1 = sv("gam1", g1n, 1.0, idet, ALU.mult, ALU.mult)
                g2a = sv("g2a", av, 1.0, r1, ALU.mult, ALU.mult)
                g2b = sv("g2b", bv, 1.0, r0, ALU.mult, ALU.mult)
                g2n = sv("g2n", g2b, -1.0, g2a, ALU.mult, ALU.add)
                gam2 = sv("gam2", g2n, 1.0, idet, ALU.mult, ALU.mult)
                s0a = sv("s0a", gam1, 1.0, gam2, ALU.mult, ALU.add)
                s0 = pnew(f"s0_{i}_{p}")
                ts(s0, s0a, -1.0, 1.0, ALU.mult, ALU.add)

                # z_{i+1} = s0*u_i + gam1*u_{i-1} + gam2*u_{i-2}
                for m in pair:
                    c = m - mm0
                    u_i = usl[i % 3][m]
                    u_1 = usl[(i - 1) % 3][m]
                    u_2 = usl[(i - 2) % 3][m]
                    nc.scalar.mul(u_2[:], u_2[:], gam2[:, c:c + 1])
                    stt(u_2[:], u_1[:], gam1[:, c:c + 1], u_2[:],
                        ALU.mult, ALU.add)
                    stt(u_2[:], u_i[:], s0[:, c:c + 1], u_2[:],
                        ALU.mult, ALU.add)
                    if i == NITER - 1:
                        for h in range(NH):
                            sl = slice(h * NW, (h + 1) * NW)
                            o32 = scratch.tile([128, NW], F32, tag="s",
                                               name=f"o32_{m}_{h}")
                            nc.scalar.copy(o32[:], u_2[:, sl])
                            nc.sync.dma_start(
                                out_d[m * 128:(m + 1) * 128, sl], o32[:])

        if NITER < MAX_ITER:
            # debug runs: dump whatever z slot is current
            zz = usl[zslot[NITER] if NITER >= 1 else 0]
            for m in range(MT):
                for h in range(NH):
                    sl = slice(h * NW, (h + 1) * NW)
                    o32 = scratch.tile([128, NW], F32, tag="s",
                                       name=f"oz{m}_{h}")
                    nc.vector.tensor_copy(o32[:], zz[m][:, sl])
                    nc.sync.dma_start(out_d[m * 128:(m + 1) * 128, sl],
                                      o32[:])

    nc.compile()
    return nc


def kernel(x_input, W_z, W_x, b):
    x_input = np.ascontiguousarray(x_input, dtype=np.float16)
    W_z = np.ascontiguousarray(W_z, dtype=np.float16)
    W_x = np.ascontiguousarray(W_x, dtype=np.float16)
    b = np.ascontiguousarray(b, dtype=np.float32)

    if "nc" not in _CACHE:
        _CACHE["nc"] = _build()
    nc = _CACHE["nc"]

    in_maps = [{
        "x": x_input[i * BC:(i + 1) * BC],
        "W_z": W_z, "W_x": W_x, "b": b,
    } for i in range(NCORES)]

    res = bass_utils.run_bass_kernel_spmd(nc, in_maps,
                                          core_ids=list(range(NCORES)),
                                          tmpdir=os.environ.get("K_TMPDIR"))
    _CACHE["res"] = res
    out = np.concatenate([res.results[i]["z_out"] for i in range(NCORES)],
                         axis=0)
    return out.astype(np.float32)


# revision 15
# speedup vs baseline: 2.4939x; 1.0002x over previous
"""DEQ block (Anderson acceleration, 6 iters, m=3) on 8 trn2 NeuronCores.

Data-parallel over batch: each core gets 512 of 4096 samples; W_z/W_x/b
replicated.  Single pass: all 512 samples stay SBUF-resident as 4 m-tiles
of 128 in fp16.  z shares the u-ring slots (z_{i+1} = s0*u_i + g1*u_{i-1}
+ g2*u_{i-2} overwrites the retiring u slot).

Per iteration: z^T k-tiles come from DMA-xbar transposes (no PE transpose
pass), matmuls are fp16 N=1024 accumulating over 16 k-tiles in PSUM with
xwx added via an identity matmul, tanh drains PSUM->fp16 g on ACT, the
per-sample dots P/Q1/Q2 use ACT-square / DVE tensor_tensor_reduce accums,
and the 2x2 regularized Anderson solve runs as ~21 fused [128,1] DVE ops.
"""

import sys

sys.path.insert(0, "/opt/trn_rl_repo")

import os
import numpy as np
from contextlib import ExitStack

import concourse.bass as bass
import concourse.tile as tile
from concourse import bacc, mybir, masks
from concourse import bass_utils

F32 = mybir.dt.float32
F16 = mybir.dt.float16
ALU = mybir.AluOpType
ACTF = mybir.ActivationFunctionType

B, D = 4096, 2048
NCORES = 8
BC = B // NCORES          # 512 samples per core
MT = BC // 128            # 4 m-tiles
KT = D // 128             # 16 k-tiles
NH = 2                    # n-halves for DMA/convert chunks
NW = D // NH              # 1024-wide loads
NS = 4                    # n-slices for matmuls
SW = D // NS              # 512-wide matmuls (sliced operands need N<=512)
MAX_ITER = 6
BETA, LAM = 0.8, 1e-4

NITER = int(os.environ.get("K_NITER", str(MAX_ITER)))

_CACHE = {}


def _build():
    nc = bacc.Bacc("TRN2", target_bir_lowering=False, debug=False,
                   num_devices=NCORES)

    x_d = nc.dram_tensor("x", [BC, D], F16, kind="ExternalInput").ap()
    wz_d = nc.dram_tensor("W_z", [D, D], F16, kind="ExternalInput").ap()
    wx_d = nc.dram_tensor("W_x", [D, D], F16, kind="ExternalInput").ap()
    b_d = nc.dram_tensor("b", [D], F32, kind="ExternalInput").ap()
    out_d = nc.dram_tensor("z_out", [BC, D], F32, kind="ExternalOutput").ap()

    with tile.TileContext(nc) as tc, ExitStack() as ctx:
        state = ctx.enter_context(tc.tile_pool(name="state", bufs=1))

        wz16 = state.tile([128, KT * D], F16, tag="wz16", name="wz16")
        gsl = [[state.tile([128, D], F16, tag=f"g{j}_{m}", name=f"g{j}_{m}")
                for m in range(MT)] for j in range(3)]
        usl = [[state.tile([128, D], F16, tag=f"u{j}_{m}", name=f"u{j}_{m}")
                for m in range(MT)] for j in range(3)]
        xwx = [state.tile([128, D], F16, tag=f"xwx{m}", name=f"xwx{m}")
               for m in range(MT)]
        zt = [state.tile([128, D], F16, tag=f"zt{m}", name=f"zt{m}")
              for m in range(MT)]
        identh = state.tile([128, 128], F16, tag="idh", name="idh")

        scratch = ctx.enter_context(tc.tile_pool(name="scratch", bufs=2))
        wtmp = ctx.enter_context(tc.tile_pool(name="wtmp", bufs=5))
        dots = ctx.enter_context(tc.tile_pool(name="dots", bufs=64))
        yps = ctx.enter_context(tc.tile_pool(name="yps", bufs=6, space="PSUM"))
        typs = ctx.enter_context(tc.tile_pool(name="typs", bufs=2, space="PSUM"))

        id32 = scratch.tile([128, NW], F32, tag="s", name="id32")
        masks.make_identity(nc, id32[:, 0:128])
        nc.vector.tensor_copy(identh[:], id32[:, 0:128])

        def stt(out, in0, scalar, in1, op0, op1):
            nc.vector.scalar_tensor_tensor(
                out=out, in0=in0, scalar=scalar, in1=in1, op0=op0, op1=op1)

        def ts(out, in0, s1, s2, op0, op1=None):
            nc.vector.tensor_scalar(out, in0, s1, s2, op0,
                                    *([op1] if op1 is not None else []))

        def dnew(nm):
            return dots.tile([128, 1], F32, tag="d", name=nm)[:]

        def transpose_into_zt(m, src, nm):
            for k in range(KT):
                tp = typs.tile([128, 128], F16, tag="tp", name=f"tp{nm}_{m}_{k}")
                nc.tensor.transpose(tp[:], src[:, k * 128:(k + 1) * 128],
                                    identh[:])
                nc.scalar.copy(zt[m][:, k * 128:(k + 1) * 128], tp[:])

        # ---------------- b -> broadcast fp16 ----------------
        # staged in the (not yet used) g1 ring slot of m=0
        b2d = b_d.rearrange("(p n) -> p n", p=1)
        b16 = gsl[1][0]
        for h in range(NH):
            b1 = scratch.tile([1, NW], F32, tag="s", name=f"b1_{h}")
            nc.sync.dma_start(b1[:], b2d[:, h * NW:(h + 1) * NW])
            bsl = scratch.tile([128, NW], F32, tag="s", name=f"bsl{h}")
            nc.gpsimd.partition_broadcast(bsl[:], b1[:])
            nc.vector.tensor_copy(b16[:, h * NW:(h + 1) * NW], bsl[:])

        # ---------------- x load (cast-DMA fp32->fp16), transpose ----------
        # x16 staged in the (not yet used) g2 ring slots
        for m in range(MT):
            nc.sync.dma_start(gsl[2][m][:], x_d[m * 128:(m + 1) * 128, :])
            transpose_into_zt(m, gsl[2][m][:], "x")

        # ---------------- xwx = x @ W_x + b ----------------
        # W_x streamed once as [128,512] fp32->fp16 cast-DMA col-chunks;
        # 4 waves of 4 live PSUM tiles
        for ns in range(NS):
            ps = [yps.tile([128, SW], F32, tag="yp", name=f"xwps{ns}_{m}")
                  for m in range(MT)]
            for k in range(KT):
                w16 = wtmp.tile([128, SW], F16, tag="w", name=f"wx16{ns}_{k}")
                nc.sync.dma_start(w16[:], wx_d[k * 128:(k + 1) * 128,
                                               ns * SW:(ns + 1) * SW])
                for m in range(MT):
                    nc.tensor.matmul(
                        ps[m][:], zt[m][:, k * 128:(k + 1) * 128],
                        w16[:], start=(k == 0), stop=(k == KT - 1))
            for m in range(MT):
                sl = slice(ns * SW, (ns + 1) * SW)
                stt(xwx[m][:, sl], ps[m][:], 1.0,
                    b16[:, sl], ALU.mult, ALU.add)

        # ---------------- W_z load (fp16 in DRAM) ----------------
        for k in range(KT):
            nc.sync.dma_start(wz16[:, k * D:(k + 1) * D],
                              wz_d[k * 128:(k + 1) * 128, :])

        # ---------------- iteration 0: z=0 ----------------
        # g0 = tanh(xwx); u0 = beta*g0; z1 aliases u0
        for m in range(MT):
            nc.scalar.activation(gsl[0][m][:], xwx[m][:], ACTF.Tanh)
            nc.vector.tensor_scalar_mul(usl[0][m][:], gsl[0][m][:], BETA)

        # z_i lives in u-ring slot zslot[i]
        zslot = [None, 0, 1, 2, 1, 2]
        hist = {}  # (kind, i, pair) -> [128,2] ap

        def pnew(nm):
            return dots.tile([128, 2], F32, tag="d", name=nm)[:]

        for i in range(1, NITER):
            gi = gsl[i % 3]
            for p in range(MT // 2):
                mm0 = 2 * p
                pair = (mm0, mm0 + 1)
                # z_i^T for this pair (PE): depends only on last iteration's
                # pair-p update, which finished during other MM work
                for m in pair:
                    transpose_into_zt(m, usl[zslot[i]][m][:], f"i{i}")

                pp = pnew(f"p{i}_{p}")
                q1 = pnew(f"q1_{i}_{p}") if i >= 2 else None
                q2 = pnew(f"q2_{i}_{p}") if i >= 3 else None

                for m in pair:
                    c = m - mm0
                    # matmul: y = z @ W_z + xwx, f = tanh(y)
                    for ns in range(NS):
                        ps = yps.tile([128, SW], F32, tag="yp",
                                      name=f"yp{i}_{m}_{ns}")
                        sl = slice(ns * SW, (ns + 1) * SW)
                        nc.tensor.matmul(ps[:], identh[:], xwx[m][:, sl],
                                         start=True, stop=False)
                        for k in range(KT):
                            nc.tensor.matmul(
                                ps[:], zt[m][:, k * 128:(k + 1) * 128],
                                wz16[:, k * D + ns * SW:k * D + (ns + 1) * SW],
                                start=False, stop=(k == KT - 1))
                        nc.scalar.activation(gi[m][:, sl], ps[:], ACTF.Tanh)

                    z = usl[zslot[i]][m]
                    g = gi[m]
                    # g = f - z ; u_i = beta*g + z (in-place over z for i>=4)
                    stt(g[:], g[:], 1.0, z[:], ALU.mult, ALU.subtract)
                    if i < NITER - 1:
                        stt(usl[i % 3][m][:], g[:], BETA, z[:],
                            ALU.mult, ALU.add)

                    # P = <g,g> on ACT (square + accum), dump into zt[m]
                    nc.scalar.activation(zt[m][:], g[:], ACTF.Square,
                                         accum_out=pp[:, c:c + 1])
                    if i >= 2:
                        nc.vector.scalar_tensor_tensor(
                            out=zt[m][:], in0=g[:], scalar=1.0,
                            in1=gsl[(i - 1) % 3][m][:],
                            op0=ALU.mult, op1=ALU.mult,
                            accum_out=q1[:, c:c + 1])
                    if i >= 3:
                        nc.vector.scalar_tensor_tensor(
                            out=zt[m][:], in0=g[:], scalar=1.0,
                            in1=gsl[(i - 2) % 3][m][:],
                            op0=ALU.mult, op1=ALU.mult,
                            accum_out=q2[:, c:c + 1])

                hist["P", i, p] = pp
                if i >= 2:
                    hist["Q1", i, p] = q1
                if i < 3:
                    continue

                # ---- 2x2 regularized Anderson solve, both m at once ----
                P = pp
                Q1 = hist["Q1", i, p]
                Q2 = q2
                S11 = hist["P", i - 1, p]
                S12 = hist["Q1", i - 1, p]
                S22 = hist["P", i - 2, p]

                def sv(nm, in0, scalar, in1, op0, op1):
                    o = pnew(f"{nm}_{i}_{p}")
                    stt(o, in0, scalar, in1, op0, op1)
                    return o

                r0 = sv("r0", Q1, -1.0, P, ALU.mult, ALU.add)
                r1 = sv("r1", Q2, -1.0, P, ALU.mult, ALU.add)
                a1 = sv("a1", Q1, -2.0, S11, ALU.mult, ALU.add)
                av = sv("av", a1, LAM, P, ALU.add, ALU.add)
                d1 = sv("d1", Q2, -2.0, S22, ALU.mult, ALU.add)
                dv = sv("dv", d1, LAM, P, ALU.add, ALU.add)
                b0 = sv("b0", Q1, 1.0, Q2, ALU.mult, ALU.add)
                b1t = sv("b1", S12, 1.0, P, ALU.mult, ALU.add)
                bv = sv("bv", b0, -1.0, b1t, ALU.mult, ALU.add)
                t4 = sv("t4", av, 1.0, dv, ALU.mult, ALU.mult)
                t5 = sv("t5", bv, 1.0, bv, ALU.mult, ALU.mult)
                d0 = sv("d0", t5, -1.0, t4, ALU.mult, ALU.add)
                det = pnew(f"det_{i}_{p}")
                ts(det, d0, 1e-8, None, ALU.add)
                idet = pnew(f"idet_{i}_{p}")
                nc.vector.reciprocal(idet, det)
                g1a = sv("g1a", dv, 1.0, r0, ALU.mult, ALU.mult)
                g1b = sv("g1b", bv, 1.0, r1, ALU.mult, ALU.mult)
                g1n = sv("g1n", g1b, -1.0, g1a, ALU.mult, ALU.add)
                gam1 = sv("gam1", g1n, 1.0, idet, ALU.mult, ALU.mult)
                g2a = sv("g2a", av, 1.0, r1, ALU.mult, ALU.mult)
                g2b = sv("g2b", bv, 1.0, r0, ALU.mult, ALU.mult)
                g2n = sv("g2n", g2b, -1.0, g2a, ALU.mult, ALU.add)
                gam2 = sv("gam2", g2n, 1.0, idet, ALU.mult, ALU.mult)
                s0a = sv("s0a", gam1, 1.0, gam2, ALU.mult, ALU.add)
                s0 = pnew(f"s0_{i}_{p}")
                ts(s0, s0a, -1.0, 1.0, ALU.mult, ALU.add)

                # z_{i+1} = s0*u_i + gam1*u_{i-1} + gam2*u_{i-2}
                if i == NITER - 1:
                    s0b = pnew(f"s0b_{i}_{p}")
                    ts(s0b, s0, BETA, None, ALU.mult)
                for m in pair:
                    c = m - mm0
                    u_1 = usl[(i - 1) % 3][m]
                    u_2 = usl[(i - 2) % 3][m]
                    nc.scalar.mul(u_2[:], u_2[:], gam2[:, c:c + 1])
                    stt(u_2[:], u_1[:], gam1[:, c:c + 1], u_2[:],
                        ALU.mult, ALU.add)
                    if i < NITER - 1:
                        u_i = usl[i % 3][m]
                        stt(u_2[:], u_i[:], s0[:, c:c + 1], u_2[:],
                            ALU.mult, ALU.add)
                    else:
                        # u_i never materialized: z6 = s0*z5 + s0b*g5 + t
                        z5 = usl[zslot[i]][m]
                        stt(u_2[:], z5[:], s0[:, c:c + 1], u_2[:],
                            ALU.mult, ALU.add)
                        stt(u_2[:], gi[m][:], s0b[:, c:c + 1], u_2[:],
                            ALU.mult, ALU.add)
                        for h in range(NH):
                            sl = slice(h * NW, (h + 1) * NW)
                            o32 = scratch.tile([128, NW], F32, tag="s",
                                               name=f"o32_{m}_{h}")
                            nc.scalar.copy(o32[:], u_2[:, sl])
                            nc.sync.dma_start(
                                out_d[m * 128:(m + 1) * 128, sl], o32[:])

        if NITER < MAX_ITER:
            # debug runs: dump whatever z slot is current
            zz = usl[zslot[NITER] if NITER >= 1 else 0]
            for m in range(MT):
                for h in range(NH):
                    sl = slice(h * NW, (h + 1) * NW)
                    o32 = scratch.tile([128, NW], F32, tag="s",
                                       name=f"oz{m}_{h}")
                    nc.vector.tensor_copy(o32[:], zz[m][:, sl])
                    nc.sync.dma_start(out_d[m * 128:(m + 1) * 128, sl],
                                      o32[:])

    nc.compile()
    return nc


def kernel(x_input, W_z, W_x, b):
    x_input = np.ascontiguousarray(x_input, dtype=np.float16)
    W_z = np.ascontiguousarray(W_z, dtype=np.float16)
    W_x = np.ascontiguousarray(W_x, dtype=np.float16)
    b = np.ascontiguousarray(b, dtype=np.float32)

    if "nc" not in _CACHE:
        _CACHE["nc"] = _build()
    nc = _CACHE["nc"]

    in_maps = [{
        "x": x_input[i * BC:(i + 1) * BC],
        "W_z": W_z, "W_x": W_x, "b": b,
    } for i in range(NCORES)]

    res = bass_utils.run_bass_kernel_spmd(nc, in_maps,
                                          core_ids=list(range(NCORES)),
                                          tmpdir=os.environ.get("K_TMPDIR"))
    _CACHE["res"] = res
    out = np.concatenate([res.results[i]["z_out"] for i in range(NCORES)],
                         axis=0)
    return out.astype(np.float32)


# revision 16
# speedup vs baseline: 2.4961x; 1.0009x over previous
"""DEQ block (Anderson acceleration, 6 iters, m=3) on 8 trn2 NeuronCores.

Data-parallel over batch: each core gets 512 of 4096 samples; W_z/W_x/b
replicated.  Single pass: all 512 samples stay SBUF-resident as 4 m-tiles
of 128 in fp16.  z shares the u-ring slots (z_{i+1} = s0*u_i + g1*u_{i-1}
+ g2*u_{i-2} overwrites the retiring u slot).

Per iteration: z^T k-tiles come from DMA-xbar transposes (no PE transpose
pass), matmuls are fp16 N=1024 accumulating over 16 k-tiles in PSUM with
xwx added via an identity matmul, tanh drains PSUM->fp16 g on ACT, the
per-sample dots P/Q1/Q2 use ACT-square / DVE tensor_tensor_reduce accums,
and the 2x2 regularized Anderson solve runs as ~21 fused [128,1] DVE ops.
"""

import sys

sys.path.insert(0, "/opt/trn_rl_repo")

import os
import numpy as np
from contextlib import ExitStack

import concourse.bass as bass
import concourse.tile as tile
from concourse import bacc, mybir, masks
from concourse import bass_utils

F32 = mybir.dt.float32
F16 = mybir.dt.float16
ALU = mybir.AluOpType
ACTF = mybir.ActivationFunctionType

B, D = 4096, 2048
NCORES = 8
BC = B // NCORES          # 512 samples per core
MT = BC // 128            # 4 m-tiles
KT = D // 128             # 16 k-tiles
NH = 2                    # n-halves for DMA/convert chunks
NW = D // NH              # 1024-wide loads
NS = 4                    # n-slices for matmuls
SW = D // NS              # 512-wide matmuls (sliced operands need N<=512)
MAX_ITER = 6
BETA, LAM = 0.8, 1e-4

NITER = int(os.environ.get("K_NITER", str(MAX_ITER)))

_CACHE = {}


def _build():
    nc = bacc.Bacc("TRN2", target_bir_lowering=False, debug=False,
                   num_devices=NCORES)

    x_d = nc.dram_tensor("x", [BC, D], F16, kind="ExternalInput").ap()
    wz_d = nc.dram_tensor("W_z", [D, D], F16, kind="ExternalInput").ap()
    wx_d = nc.dram_tensor("W_x", [D, D], F16, kind="ExternalInput").ap()
    b_d = nc.dram_tensor("b", [D], F32, kind="ExternalInput").ap()
    out_d = nc.dram_tensor("z_out", [BC, D], F32, kind="ExternalOutput").ap()

    with tile.TileContext(nc) as tc, ExitStack() as ctx:
        state = ctx.enter_context(tc.tile_pool(name="state", bufs=1))

        wz16 = state.tile([128, KT * D], F16, tag="wz16", name="wz16")
        gsl = [[state.tile([128, D], F16, tag=f"g{j}_{m}", name=f"g{j}_{m}")
                for m in range(MT)] for j in range(3)]
        usl = [[state.tile([128, D], F16, tag=f"u{j}_{m}", name=f"u{j}_{m}")
                for m in range(MT)] for j in range(3)]
        xwx = [state.tile([128, D], F16, tag=f"xwx{m}", name=f"xwx{m}")
               for m in range(MT)]
        zt = [state.tile([128, D], F16, tag=f"zt{m}", name=f"zt{m}")
              for m in range(MT)]
        identh = state.tile([128, 128], F16, tag="idh", name="idh")

        scratch = ctx.enter_context(tc.tile_pool(name="scratch", bufs=2))
        wtmp = ctx.enter_context(tc.tile_pool(name="wtmp", bufs=5))
        dots = ctx.enter_context(tc.tile_pool(name="dots", bufs=64))
        yps = ctx.enter_context(tc.tile_pool(name="yps", bufs=6, space="PSUM"))
        typs = ctx.enter_context(tc.tile_pool(name="typs", bufs=2, space="PSUM"))

        id32 = scratch.tile([128, NW], F32, tag="s", name="id32")
        masks.make_identity(nc, id32[:, 0:128])
        nc.vector.tensor_copy(identh[:], id32[:, 0:128])

        def stt(out, in0, scalar, in1, op0, op1):
            nc.vector.scalar_tensor_tensor(
                out=out, in0=in0, scalar=scalar, in1=in1, op0=op0, op1=op1)

        def ts(out, in0, s1, s2, op0, op1=None):
            nc.vector.tensor_scalar(out, in0, s1, s2, op0,
                                    *([op1] if op1 is not None else []))

        def dnew(nm):
            return dots.tile([128, 1], F32, tag="d", name=nm)[:]

        def transpose_into_zt(m, src, nm):
            for k in range(KT):
                tp = typs.tile([128, 128], F16, tag="tp", name=f"tp{nm}_{m}_{k}")
                nc.tensor.transpose(tp[:], src[:, k * 128:(k + 1) * 128],
                                    identh[:])
                nc.scalar.copy(zt[m][:, k * 128:(k + 1) * 128], tp[:])

        # ---------------- b -> broadcast fp16 ----------------
        # staged in the (not yet used) g1 ring slot of m=0
        b2d = b_d.rearrange("(p n) -> p n", p=1)
        b16 = gsl[1][0]
        for h in range(NH):
            b1 = scratch.tile([1, NW], F32, tag="s", name=f"b1_{h}")
            nc.sync.dma_start(b1[:], b2d[:, h * NW:(h + 1) * NW])
            bsl = scratch.tile([128, NW], F32, tag="s", name=f"bsl{h}")
            nc.gpsimd.partition_broadcast(bsl[:], b1[:])
            nc.vector.tensor_copy(b16[:, h * NW:(h + 1) * NW], bsl[:])

        # ---------------- x load (cast-DMA fp32->fp16), transpose ----------
        # x16 staged in the (not yet used) g2 ring slots
        for m in range(MT):
            nc.sync.dma_start(gsl[2][m][:], x_d[m * 128:(m + 1) * 128, :])
            transpose_into_zt(m, gsl[2][m][:], "x")

        # ---------------- xwx = x @ W_x + b ----------------
        # W_x streamed once as [128,512] fp32->fp16 cast-DMA col-chunks;
        # 4 waves of 4 live PSUM tiles
        for ns in range(NS):
            ps = [yps.tile([128, SW], F32, tag="yp", name=f"xwps{ns}_{m}")
                  for m in range(MT)]
            for k in range(KT):
                w16 = wtmp.tile([128, SW], F16, tag="w", name=f"wx16{ns}_{k}")
                nc.sync.dma_start(w16[:], wx_d[k * 128:(k + 1) * 128,
                                               ns * SW:(ns + 1) * SW])
                for m in range(MT):
                    nc.tensor.matmul(
                        ps[m][:], zt[m][:, k * 128:(k + 1) * 128],
                        w16[:], start=(k == 0), stop=(k == KT - 1))
            for m in range(MT):
                sl = slice(ns * SW, (ns + 1) * SW)
                stt(xwx[m][:, sl], ps[m][:], 1.0,
                    b16[:, sl], ALU.mult, ALU.add)
                # iteration 0 slice-wise: g0 = tanh(xwx), u0 = beta*g0;
                # lets iteration 1's transposes/matmuls start per k-tile
                nc.scalar.activation(gsl[0][m][:, sl], xwx[m][:, sl],
                                     ACTF.Tanh)
                nc.vector.tensor_scalar_mul(usl[0][m][:, sl],
                                            gsl[0][m][:, sl], BETA)

        # ---------------- W_z load (fp16 in DRAM) ----------------
        for k in range(KT):
            nc.sync.dma_start(wz16[:, k * D:(k + 1) * D],
                              wz_d[k * 128:(k + 1) * 128, :])

        # z_i lives in u-ring slot zslot[i]
        zslot = [None, 0, 1, 2, 1, 2]
        hist = {}  # (kind, i, pair) -> [128,2] ap

        def pnew(nm):
            return dots.tile([128, 2], F32, tag="d", name=nm)[:]

        for i in range(1, NITER):
            gi = gsl[i % 3]
            for p in range(MT // 2):
                mm0 = 2 * p
                pair = (mm0, mm0 + 1)
                # z_i^T for this pair (PE): depends only on last iteration's
                # pair-p update, which finished during other MM work
                for m in pair:
                    transpose_into_zt(m, usl[zslot[i]][m][:], f"i{i}")

                pp = pnew(f"p{i}_{p}")
                q1 = pnew(f"q1_{i}_{p}") if i >= 2 else None
                q2 = pnew(f"q2_{i}_{p}") if i >= 3 else None

                for m in pair:
                    c = m - mm0
                    # matmul: y = z @ W_z + xwx, f = tanh(y)
                    for ns in range(NS):
                        ps = yps.tile([128, SW], F32, tag="yp",
                                      name=f"yp{i}_{m}_{ns}")
                        sl = slice(ns * SW, (ns + 1) * SW)
                        nc.tensor.matmul(ps[:], identh[:], xwx[m][:, sl],
                                         start=True, stop=False)
                        for k in range(KT):
                            nc.tensor.matmul(
                                ps[:], zt[m][:, k * 128:(k + 1) * 128],
                                wz16[:, k * D + ns * SW:k * D + (ns + 1) * SW],
                                start=False, stop=(k == KT - 1))
                        nc.scalar.activation(gi[m][:, sl], ps[:], ACTF.Tanh)

                    z = usl[zslot[i]][m]
                    g = gi[m]
                    # g = f - z ; u_i = beta*g + z (in-place over z for i>=4)
                    stt(g[:], g[:], 1.0, z[:], ALU.mult, ALU.subtract)
                    if i < NITER - 1:
                        stt(usl[i % 3][m][:], g[:], BETA, z[:],
                            ALU.mult, ALU.add)

                    # P = <g,g> on ACT (square + accum), dump into zt[m]
                    nc.scalar.activation(zt[m][:], g[:], ACTF.Square,
                                         accum_out=pp[:, c:c + 1])
                    if i >= 2:
                        nc.vector.scalar_tensor_tensor(
                            out=zt[m][:], in0=g[:], scalar=1.0,
                            in1=gsl[(i - 1) % 3][m][:],
                            op0=ALU.mult, op1=ALU.mult,
                            accum_out=q1[:, c:c + 1])
                    if i >= 3:
                        nc.vector.scalar_tensor_tensor(
                            out=zt[m][:], in0=g[:], scalar=1.0,
                            in1=gsl[(i - 2) % 3][m][:],
                            op0=ALU.mult, op1=ALU.mult,
                            accum_out=q2[:, c:c + 1])

                hist["P", i, p] = pp
                if i >= 2:
                    hist["Q1", i, p] = q1
                if i < 3:
                    continue

                # ---- 2x2 regularized Anderson solve, both m at once ----
                P = pp
                Q1 = hist["Q1", i, p]
                Q2 = q2
                S11 = hist["P", i - 1, p]
                S12 = hist["Q1", i - 1, p]
                S22 = hist["P", i - 2, p]

                def sv(nm, in0, scalar, in1, op0, op1):
                    o = pnew(f"{nm}_{i}_{p}")
                    stt(o, in0, scalar, in1, op0, op1)
                    return o

                r0 = sv("r0", Q1, -1.0, P, ALU.mult, ALU.add)
                r1 = sv("r1", Q2, -1.0, P, ALU.mult, ALU.add)
                a1 = sv("a1", Q1, -2.0, S11, ALU.mult, ALU.add)
                av = sv("av", a1, LAM, P, ALU.add, ALU.add)
                d1 = sv("d1", Q2, -2.0, S22, ALU.mult, ALU.add)
                dv = sv("dv", d1, LAM, P, ALU.add, ALU.add)
                b0 = sv("b0", Q1, 1.0, Q2, ALU.mult, ALU.add)
                b1t = sv("b1", S12, 1.0, P, ALU.mult, ALU.add)
                bv = sv("bv", b0, -1.0, b1t, ALU.mult, ALU.add)
                t4 = sv("t4", av, 1.0, dv, ALU.mult, ALU.mult)
                t5 = sv("t5", bv, 1.0, bv, ALU.mult, ALU.mult)
                d0 = sv("d0", t5, -1.0, t4, ALU.mult, ALU.add)
                det = pnew(f"det_{i}_{p}")
                ts(det, d0, 1e-8, None, ALU.add)
                idet = pnew(f"idet_{i}_{p}")
                nc.vector.reciprocal(idet, det)
                g1a = sv("g1a", dv, 1.0, r0, ALU.mult, ALU.mult)
                g1b = sv("g1b", bv, 1.0, r1, ALU.mult, ALU.mult)
                g1n = sv("g1n", g1b, -1.0, g1a, ALU.mult, ALU.add)
                gam1 = sv("gam1", g1n, 1.0, idet, ALU.mult, ALU.mult)
                g2a = sv("g2a", av, 1.0, r1, ALU.mult, ALU.mult)
                g2b = sv("g2b", bv, 1.0, r0, ALU.mult, ALU.mult)
                g2n = sv("g2n", g2b, -1.0, g2a, ALU.mult, ALU.add)
                gam2 = sv("gam2", g2n, 1.0, idet, ALU.mult, ALU.mult)
                s0a = sv("s0a", gam1, 1.0, gam2, ALU.mult, ALU.add)
                s0 = pnew(f"s0_{i}_{p}")
                ts(s0, s0a, -1.0, 1.0, ALU.mult, ALU.add)

                # z_{i+1} = s0*u_i + gam1*u_{i-1} + gam2*u_{i-2}
                if i == NITER - 1:
                    s0b = pnew(f"s0b_{i}_{p}")
                    ts(s0b, s0, BETA, None, ALU.mult)
                for m in pair:
                    c = m - mm0
                    u_1 = usl[(i - 1) % 3][m]
                    u_2 = usl[(i - 2) % 3][m]
                    nc.scalar.mul(u_2[:], u_2[:], gam2[:, c:c + 1])
                    stt(u_2[:], u_1[:], gam1[:, c:c + 1], u_2[:],
                        ALU.mult, ALU.add)
                    if i < NITER - 1:
                        u_i = usl[i % 3][m]
                        stt(u_2[:], u_i[:], s0[:, c:c + 1], u_2[:],
                            ALU.mult, ALU.add)
                    else:
                        # u_i never materialized: z6 = s0*z5 + s0b*g5 + t
                        z5 = usl[zslot[i]][m]
                        stt(u_2[:], z5[:], s0[:, c:c + 1], u_2[:],
                            ALU.mult, ALU.add)
                        stt(u_2[:], gi[m][:], s0b[:, c:c + 1], u_2[:],
                            ALU.mult, ALU.add)
                        for h in range(NH):
                            sl = slice(h * NW, (h + 1) * NW)
                            o32 = scratch.tile([128, NW], F32, tag="s",
                                               name=f"o32_{m}_{h}")
                            nc.scalar.copy(o32[:], u_2[:, sl])
                            nc.sync.dma_start(
                                out_d[m * 128:(m + 1) * 128, sl], o32[:])

        if NITER < MAX_ITER:
            # debug runs: dump whatever z slot is current
            zz = usl[zslot[NITER] if NITER >= 1 else 0]
            for m in range(MT):
                for h in range(NH):
                    sl = slice(h * NW, (h + 1) * NW)
                    o32 = scratch.tile([128, NW], F32, tag="s",
                                       name=f"oz{m}_{h}")
                    nc.vector.tensor_copy(o32[:], zz[m][:, sl])
                    nc.sync.dma_start(out_d[m * 128:(m + 1) * 128, sl],
                                      o32[:])

    nc.compile()
    return nc


def kernel(x_input, W_z, W_x, b):
    x_input = np.ascontiguousarray(x_input, dtype=np.float16)
    W_z = np.ascontiguousarray(W_z, dtype=np.float16)
    W_x = np.ascontiguousarray(W_x, dtype=np.float16)
    b = np.ascontiguousarray(b, dtype=np.float32)

    if "nc" not in _CACHE:
        _CACHE["nc"] = _build()
    nc = _CACHE["nc"]

    in_maps = [{
        "x": x_input[i * BC:(i + 1) * BC],
        "W_z": W_z, "W_x": W_x, "b": b,
    } for i in range(NCORES)]

    res = bass_utils.run_bass_kernel_spmd(nc, in_maps,
                                          core_ids=list(range(NCORES)),
                                          tmpdir=os.environ.get("K_TMPDIR"))
    _CACHE["res"] = res
    out = np.concatenate([res.results[i]["z_out"] for i in range(NCORES)],
                         axis=0)
    return out.astype(np.float32)
